# revision 5
# baseline (speedup 1.0000x reference)
"""Trainium2 Bass kernel for nn_DynamicModel_79955111182517 (DGCNN-style
EdgeConv GNN, B=8 graphs x P=512 points x C=1024, k=16 kNN).

Strategy (data-parallel, one graph per NeuronCore):
- BatchNorm (eval) scale/shift folded into adjacent matmuls on the host;
  max-aggregations commute with the positive per-channel scales.
- EdgeConv message nn([x_i, x_j - x_i]) is decomposed: feat @ W =
  x_i @ (Wtop - Wbot) + x_j @ Wbot, so only two P x C matmuls are needed
  instead of P*k x 2C.
- conv1 (single-layer MLP) factorizes fully: max_k relu(c_i + b_j) =
  relu(c_i + max_{j in knn(i)} b_j).
- kNN: fp32 Gram matrix on the PE, top-16 per row via the DVE max8 /
  max_index / match_replace ISA ops; neighbor gathers are one-hot fp16
  matmuls on the PE (keeps everything feature-major for the MLP chain).
- All heavy matmuls run in fp16 (full PE rate, ~8x finer rounding than
  bf16); distances in fp32 so neighbor selection matches the reference.
"""

import sys
import numpy as np

try:
    import concourse.bass as bass  # noqa: F401
except ImportError:
    sys.path.insert(0, "/opt/trn_rl_repo")

import concourse.bass as bass
import concourse.tile as tile
from concourse import bacc, mybir
from concourse.bass_utils import run_bass_kernel_spmd

F32 = mybir.dt.float32
F16 = mybir.dt.float16
U32 = mybir.dt.uint32
AF = mybir.ActivationFunctionType
OP = mybir.AluOpType

N_CORES = 8
P = 512          # points per graph
C0 = 1024        # input channels
K = 16           # neighbors
NI = P // 128    # 4 token tiles
CH = 512         # gather-chunk columns (32 points x 16 ranks)
NCH = (P * K) // CH  # 16 chunks
NEG = -1.0e30

_PROG_CACHE = {}


# --------------------------------------------------------------------------
# device program
# --------------------------------------------------------------------------

def _declare(nc):
    t = {}

    def inp(name, shape, dt):
        t[name] = nc.dram_tensor(name, list(shape), dt, kind="ExternalInput").ap()

    inp("xg", [P, C0], F32)
    inp("wc1", [1024, 512], F16)
    inp("wb1", [1024, 512], F16)
    inp("b1v", [512], F32)
    inp("g1v", [512], F32)
    inp("bt1v", [512], F32)
    inp("wc2", [512, 512], F16)
    inp("wb2", [512, 512], F16)
    inp("b21v", [512], F32)
    inp("w22", [512, 1024], F16)
    inp("b22v", [1024], F32)
    inp("w23", [1024, 1024], F16)
    inp("b23v", [1024], F32)
    inp("wlt", [512, 2048], F16)
    inp("wlb", [1024, 2048], F16)
    inp("blv", [2048], F32)
    inp("wh1", [2048, 1024], F16)
    inp("bh1v", [1024], F32)
    inp("wh2", [1024, 512], F16)
    inp("bh2v", [512], F32)
    inp("whw", [512, 10], F16)
    inp("bwv", [10], F32)
    inp("jidx", [512], F32)         # arange(512) for one-hot compares
    inp("ident", [128, 128], F32)   # PE-transpose identity

    t["out"] = nc.dram_tensor("out", [10], F32, kind="ExternalOutput").ap()
    # internal DRAM bounce buffers (partition -> row reshuffles)
    t["irow1"] = nc.dram_tensor("irow1", [P * K], F16).ap()
    t["irow2"] = nc.dram_tensor("irow2", [P * K], F16).ap()
    t["xrow1"] = nc.dram_tensor("xrow1", [P], F32).ap()
    t["xrow2"] = nc.dram_tensor("xrow2", [P], F32).ap()
    return t


def _emit(ctx, tc, t):
    nc = tc.nc

    wa = ctx.enter_context(tc.tile_pool(name="wa", bufs=1))
    wb = ctx.enter_context(tc.tile_pool(name="wb", bufs=1))
    sb = ctx.enter_context(tc.tile_pool(name="sb", bufs=1))
    cols = ctx.enter_context(tc.tile_pool(name="cols", bufs=1))
    pmm = ctx.enter_context(tc.tile_pool(name="pmm", bufs=6, space="PSUM"))
    ptr = ctx.enter_context(tc.tile_pool(name="ptr", bufs=2, space="PSUM"))

    def col_load(vec_ap, n128, base_name, dt=F32):
        out = []
        for i in range(n128):
            n = min(128, vec_ap.shape[0] - i * 128)
            c = cols.tile([n, 1], dt, name=f"{base_name}{i}", tag=f"{base_name}{i}")
            nc.sync.dma_start(out=c, in_=vec_ap[i * 128:i * 128 + n])
            out.append(c)
        return out

    # ---- constant columns ----
    b1c = col_load(t["b1v"], 4, "b1c")
    g1c = col_load(t["g1v"], 4, "g1c")
    bt1c = col_load(t["bt1v"], 4, "bt1c")
    b21c = col_load(t["b21v"], 4, "b21c")
    b22c = col_load(t["b22v"], 8, "b22c")
    b23c = col_load(t["b23v"], 8, "b23c")
    blc = col_load(t["blv"], 16, "blc")
    bh1c = col_load(t["bh1v"], 8, "bh1c")
    bh2c = col_load(t["bh2v"], 4, "bh2c")
    bwc = col_load(t["bwv"], 1, "bwc")
    jc = col_load(t["jidx"], 4, "jc", dt=F32)
    identw = cols.tile([128, 128], F32, name="identw", tag="identw")
    nc.sync.dma_start(out=identw, in_=t["ident"])

    # ---- weight slab A: conv1 (later reused for lin1) ----
    # layout: [wc1 8kt x 512 | wb1 8kt x 512]
    slabA = wa.tile([128, 16 * 512], F16, name="slabA", tag="wslabA")
    for kt in range(8):
        nc.sync.dma_start(out=slabA[:, kt * 512:(kt + 1) * 512],
                          in_=t["wc1"][kt * 128:(kt + 1) * 128, :])
        nc.sync.dma_start(out=slabA[:, (8 + kt) * 512:(9 + kt) * 512],
                          in_=t["wb1"][kt * 128:(kt + 1) * 128, :])

    def wA1(kt, mt):  # wc1 lhsT block [128k, 128m]
        return slabA[:, kt * 512 + mt * 128: kt * 512 + (mt + 1) * 128]

    def wA2(kt):      # wb1 rhs k-tile [128k, 512]
        return slabA[:, (8 + kt) * 512:(9 + kt) * 512]

    # ---- weight slab B: conv2 (later reused for heads) ----
    # layout: [wc2 4x512 | wb2 4x512 | w22 4x1024 | w23 8x1024]
    slabB = wb.tile([128, 2048 + 2048 + 4096 + 8192], F16, name="slabB", tag="wslabB")
    oB2 = 2048
    oB3 = 4096
    oB4 = 8192
    for kt in range(4):
        nc.sync.dma_start(out=slabB[:, kt * 512:(kt + 1) * 512],
                          in_=t["wc2"][kt * 128:(kt + 1) * 128, :])
        nc.sync.dma_start(out=slabB[:, oB2 + kt * 512:oB2 + (kt + 1) * 512],
                          in_=t["wb2"][kt * 128:(kt + 1) * 128, :])
        nc.sync.dma_start(out=slabB[:, oB3 + kt * 1024:oB3 + (kt + 1) * 1024],
                          in_=t["w22"][kt * 128:(kt + 1) * 128, :])
    for kt in range(8):
        nc.sync.dma_start(out=slabB[:, oB4 + kt * 1024:oB4 + (kt + 1) * 1024],
                          in_=t["w23"][kt * 128:(kt + 1) * 128, :])

    def wB_c2(kt, mt):
        return slabB[:, kt * 512 + mt * 128: kt * 512 + (mt + 1) * 128]

    def wB_b2(kt):
        return slabB[:, oB2 + kt * 512:oB2 + (kt + 1) * 512]

    def wB_22(kt, nt):
        return slabB[:, oB3 + kt * 1024 + nt * 128: oB3 + kt * 1024 + (nt + 1) * 128]

    def wB_23(kt, nt):
        return slabB[:, oB4 + kt * 1024 + nt * 128: oB4 + kt * 1024 + (nt + 1) * 128]

    # ==================================================================
    # stage 0/1: load X blockwise, transpose on PE -> XT (fp32 + fp16)
    # ==================================================================
    xt32 = [sb.tile([128, P], F32, name=f"xt32_{ct}", tag=f"xt32_{ct}") for ct in range(8)]
    xt16 = [sb.tile([128, P], F16, name=f"xt16_{ct}", tag=f"xt16_{ct}") for ct in range(8)]
    for ct in range(8):
        for it in range(NI):
            xb = sb.tile([128, 128], F32, name="xb", tag="xblk", bufs=6)
            nc.sync.dma_start(
                out=xb, in_=t["xg"][it * 128:(it + 1) * 128, ct * 128:(ct + 1) * 128])
            pt = ptr.tile([128, 128], F32, name="pt", tag="ptr")
            nc.tensor.transpose(pt, xb, identw)
            nc.scalar.activation(out=xt32[ct][:, it * 128:(it + 1) * 128], in_=pt, func=AF.Copy)
        nc.vector.tensor_copy(out=xt16[ct], in_=xt32[ct])

    # ==================================================================
    # generic helpers
    # ==================================================================
    def gram_topk_idxrow(xt_f32, nk, gsb_tag, irow_dram, xrow_dram, conv_id):
        """Gram matrix (fp32), scores s = 2G - x2_j, top-16 indices per row,
        assembled into a broadcast index row [128, P*K] (f16 values)."""
        gsb = []
        for mt in range(NI):
            pg = pmm.tile([128, P], F32, name="pg", tag="pmm")
            for kt in range(nk):
                nc.tensor.matmul(pg, lhsT=xt_f32[kt][:, mt * 128:(mt + 1) * 128],
                                 rhs=xt_f32[kt], start=(kt == 0), stop=(kt == nk - 1))
            g = sb.tile([128, P], F32, name=f"g{conv_id}_{mt}", tag=f"{gsb_tag}{mt}")
            nc.scalar.activation(out=g, in_=pg, func=AF.Copy)
            gsb.append(g)

        # x2 row from the diagonal of G
        for mt in range(NI):
            dsel = sb.tile([128, P], F32, name="dsel", tag="dsel", bufs=2)
            nc.gpsimd.affine_select(
                out=dsel, in_=gsb[mt], compare_op=OP.is_equal, fill=0.0,
                base=128 * mt, pattern=[[-1, P]], channel_multiplier=1)
            dcol = sb.tile([128, 1], F32, name="dcol", tag="dcol", bufs=2)
            nc.vector.reduce_sum(out=dcol, in_=dsel, axis=mybir.AxisListType.X)
            nc.sync.dma_start(out=xrow_dram[mt * 128:(mt + 1) * 128], in_=dcol)
        x2b = sb.tile([128, P], F32, name=f"x2b{conv_id}", tag="x2b")
        nc.sync.dma_start(out=x2b, in_=xrow_dram.rearrange("(o n) -> o n", o=1).to_broadcast([128, P]))

        # s = 2G - x2_j (in place over gsb); then top-16 via max8/match_replace
        for mt in range(NI):
            nc.vector.scalar_tensor_tensor(
                out=gsb[mt], in0=gsb[mt], scalar=2.0, in1=x2b,
                op0=OP.mult, op1=OP.subtract)
            va = sb.tile([128, 8], F32, name="va", tag="va", bufs=2)
            vb = sb.tile([128, 8], F32, name="vb", tag="vb", bufs=2)
            idxu = sb.tile([128, K], U32, name="idxu", tag="idxu", bufs=2)
            idxf = sb.tile([128, K], F16, name="idxf", tag="idxf", bufs=2)
            nc.vector.max(out=va, in_=gsb[mt])
            nc.vector.max_index(out=idxu[:, 0:8], in_max=va, in_values=gsb[mt])
            nc.vector.match_replace(out=gsb[mt], in_to_replace=va,
                                    in_values=gsb[mt], imm_value=NEG)
            nc.vector.max(out=vb, in_=gsb[mt])
            nc.vector.max_index(out=idxu[:, 8:16], in_max=vb, in_values=gsb[mt])
            nc.vector.tensor_copy(out=idxf, in_=idxu)
            nc.sync.dma_start(out=irow_dram[mt * 128 * K:(mt + 1) * 128 * K], in_=idxf)

        irow_b = sb.tile([128, P * K], F16, name=f"irow_b{conv_id}", tag="irow_b")
        nc.sync.dma_start(out=irow_b, in_=irow_dram.rearrange("(o n) -> o n", o=1).to_broadcast([128, P * K]))
        return irow_b

    def onehot_chunk(irow_b, ch, conv_id):
        ohs = []
        for jt in range(NI):
            oh = sb.tile([128, CH], F16, name="oh", tag=f"oh{jt}", bufs=2)
            nc.vector.tensor_scalar(
                oh, irow_b[:, ch * CH:(ch + 1) * CH], jc[jt], None, op0=OP.is_equal)
            ohs.append(oh)
        return ohs

    # ==================================================================
    # conv1
    # ==================================================================
    irow_b1 = gram_topk_idxrow(xt32, 8, "gsb", t["irow1"], t["xrow1"], 1)

    # c1 = X @ (Wtop-Wbot) + b1  (feature-major), b1tok = X @ Wbot (token-major)
    c1T = []
    for mt in range(NI):
        pc = pmm.tile([128, P], F32, name="pc", tag="pmm")
        for kt in range(8):
            nc.tensor.matmul(pc, lhsT=wA1(kt, mt), rhs=xt16[kt],
                             start=(kt == 0), stop=(kt == 7))
        c = sb.tile([128, P], F32, name=f"c1T{mt}", tag=f"cT{mt}")
        nc.vector.tensor_scalar(c, pc, b1c[mt], None, op0=OP.add)
        c1T.append(c)
    btok1 = []
    for jt in range(NI):
        pb = pmm.tile([128, 512], F32, name="pb", tag="pmm")
        for kt in range(8):
            nc.tensor.matmul(pb, lhsT=xt16[kt][:, jt * 128:(jt + 1) * 128],
                             rhs=wA2(kt), start=(kt == 0), stop=(kt == 7))
        bt_ = sb.tile([128, 512], F16, name=f"btok1{jt}", tag=f"btok{jt}")
        nc.vector.tensor_copy(out=bt_, in_=pb)
        btok1.append(bt_)

    # gather chunks: T[c, i] = max_r b[knn_r(i), c]
    Tt = [sb.tile([128, P], F32, name=f"Tt{ct}", tag=f"xt32_{ct}") for ct in range(NI)]
    for ch in range(NCH):
        ohs = onehot_chunk(irow_b1, ch, 1)
        for ct in range(NI):
            pgt = pmm.tile([128, CH], F32, name="pgt", tag="pmm")
            for jt in range(NI):
                nc.tensor.matmul(pgt, lhsT=btok1[jt][:, ct * 128:(ct + 1) * 128],
                                 rhs=ohs[jt], start=(jt == 0), stop=(jt == NI - 1))
            nc.vector.tensor_reduce(
                out=Tt[ct][:, ch * (CH // K):(ch + 1) * (CH // K)],
                in_=pgt.rearrange("p (i r) -> p i r", r=K),
                axis=mybir.AxisListType.X, op=OP.max)

    # x1 = g1 * relu(c1 + T) + bt1   (keep fp32 for Gram, fp16 for matmuls)
    x1_32 = []
    x1_16 = []
    for ct in range(NI):
        nc.vector.tensor_add(c1T[ct], c1T[ct], Tt[ct])
        x32 = sb.tile([128, P], F32, name=f"x1_32_{ct}", tag=f"xt32_{4 + ct}")
        nc.scalar.activation(out=x32, in_=c1T[ct], func=AF.Relu)
        nc.vector.tensor_scalar(x32, x32, g1c[ct], bt1c[ct], op0=OP.mult, op1=OP.add)
        x16 = sb.tile([128, P], F16, name=f"x1_16_{ct}", tag=f"x1_16_{ct}")
        nc.vector.tensor_copy(out=x16, in_=x32)
        x1_32.append(x32)
        x1_16.append(x16)

    # ==================================================================
    # conv2
    # ==================================================================
    irow_b2 = gram_topk_idxrow(x1_32, NI, "gsb", t["irow2"], t["xrow2"], 2)

    c2T = []
    for mt in range(NI):
        pc2 = pmm.tile([128, P], F32, name="pc2", tag="pmm")
        for kt in range(NI):
            nc.tensor.matmul(pc2, lhsT=wB_c2(kt, mt), rhs=x1_16[kt],
                             start=(kt == 0), stop=(kt == NI - 1))
        c2 = sb.tile([128, P], F32, name=f"c2T{mt}", tag=f"cT{mt}")
        nc.vector.tensor_scalar(c2, pc2, b21c[mt], None, op0=OP.add)
        c2T.append(c2)
    btok2 = []
    for jt in range(NI):
        pb2 = pmm.tile([128, 512], F32, name="pb2", tag="pmm")
        for kt in range(NI):
            nc.tensor.matmul(pb2, lhsT=x1_16[kt][:, jt * 128:(jt + 1) * 128],
                             rhs=wB_b2(kt), start=(kt == 0), stop=(kt == NI - 1))
        bt2 = sb.tile([128, 512], F16, name=f"btok2{jt}", tag=f"btok{jt}")
        nc.vector.tensor_copy(out=bt2, in_=pb2)
        btok2.append(bt2)

    x2acc = [sb.tile([128, P], F32, name=f"x2acc{nt}", tag=f"xt32_{nt}") for nt in range(8)]
    IC = CH // K  # 32 points per chunk
    for ch in range(NCH):
        ohs = onehot_chunk(irow_b2, ch, 2)
        # gather + add c2 (broadcast over ranks) + relu -> h1 (fp16)
        h1 = []
        for ct in range(NI):
            pg2 = pmm.tile([128, CH], F32, name="pg2", tag="pmm")
            for jt in range(NI):
                nc.tensor.matmul(pg2, lhsT=btok2[jt][:, ct * 128:(ct + 1) * 128],
                                 rhs=ohs[jt], start=(jt == 0), stop=(jt == NI - 1))
            h = sb.tile([128, CH], F16, name="h1", tag=f"h1_{ct}", bufs=2)
            cbc = c2T[ct][:, ch * IC:(ch + 1) * IC].rearrange(
                "p (i o) -> p i o", o=1).to_broadcast([128, IC, K])
            nc.vector.tensor_tensor(
                out=h.rearrange("p (i r) -> p i r", r=K),
                in0=pg2.rearrange("p (i r) -> p i r", r=K),
                in1=cbc, op=OP.add)
            nc.scalar.activation(out=h, in_=h, func=AF.Relu)
            h1.append(h)
        # layer2: h2 = relu(h1 @ W22f + b22f)
        h2 = []
        for nt in range(8):
            pz2 = pmm.tile([128, CH], F32, name="pz2", tag="pmm")
            for kt in range(NI):
                nc.tensor.matmul(pz2, lhsT=wB_22(kt, nt), rhs=h1[kt],
                                 start=(kt == 0), stop=(kt == NI - 1))
            hh = sb.tile([128, CH], F16, name="h2", tag=f"h2_{nt}", bufs=2)
            nc.scalar.activation(out=hh, in_=pz2, func=AF.Relu, bias=b22c[nt])
            h2.append(hh)
        # layer3 + max over ranks
        for nt in range(8):
            pz3 = pmm.tile([128, CH], F32, name="pz3", tag="pmm")
            for kt in range(8):
                nc.tensor.matmul(pz3, lhsT=wB_23(kt, nt), rhs=h2[kt],
                                 start=(kt == 0), stop=(kt == 7))
            nc.vector.tensor_reduce(
                out=x2acc[nt][:, ch * IC:(ch + 1) * IC],
                in_=pz3.rearrange("p (i r) -> p i r", r=K),
                axis=mybir.AxisListType.X, op=OP.max)

    # x2relu = relu(x2acc + b23f) -> fp16
    x2r = []
    for nt in range(8):
        xx = sb.tile([128, P], F16, name=f"x2r{nt}", tag=f"x2r{nt}")
        nc.scalar.activation(out=xx, in_=x2acc[nt], func=AF.Relu, bias=b23c[nt])
        x2r.append(xx)

    # ---- weight slab C: lin1 (reuses slab A space) ----
    slabC = wa.tile([128, 12 * 2048], F16, name="slabC", tag="wslabA")
    for kt in range(4):
        nc.sync.dma_start(out=slabC[:, kt * 2048:(kt + 1) * 2048],
                          in_=t["wlt"][kt * 128:(kt + 1) * 128, :])
    for kt in range(8):
        nc.sync.dma_start(out=slabC[:, (4 + kt) * 2048:(5 + kt) * 2048],
                          in_=t["wlb"][kt * 128:(kt + 1) * 128, :])

    # lin1 + global max pool -> pooled [2048] (fp16 cols)
    pooled = []
    for mt in range(16):
        pzl = pmm.tile([128, P], F32, name="pzl", tag="pmm")
        for kt in range(4):
            nc.tensor.matmul(
                pzl, lhsT=slabC[:, kt * 2048 + mt * 128: kt * 2048 + (mt + 1) * 128],
                rhs=x1_16[kt], start=(kt == 0), stop=False)
        for kt in range(8):
            nc.tensor.matmul(
                pzl, lhsT=slabC[:, (4 + kt) * 2048 + mt * 128: (4 + kt) * 2048 + (mt + 1) * 128],
                rhs=x2r[kt], start=False, stop=(kt == 7))
        pcol = sb.tile([128, 1], F32, name="pcol", tag="pcol", bufs=4)
        nc.vector.tensor_reduce(out=pcol, in_=pzl, axis=mybir.AxisListType.X, op=OP.max)
        pc16 = sb.tile([128, 1], F16, name=f"pool{mt}", tag=f"pool{mt}")
        nc.scalar.activation(out=pc16, in_=pcol, func=AF.Relu, bias=blc[mt])
        pooled.append(pc16)

    # ---- weight slab D: heads (reuses slab B space) ----
    slabD = wb.tile([128, 16 * 1024 + 8 * 512 + 4 * 16], F16, name="slabD", tag="wslabB")
    oD2 = 16 * 1024
    oD3 = oD2 + 8 * 512
    for kt in range(16):
        nc.sync.dma_start(out=slabD[:, kt * 1024:(kt + 1) * 1024],
                          in_=t["wh1"][kt * 128:(kt + 1) * 128, :])
    for kt in range(8):
        nc.sync.dma_start(out=slabD[:, oD2 + kt * 512:oD2 + (kt + 1) * 512],
                          in_=t["wh2"][kt * 128:(kt + 1) * 128, :])
    for kt in range(4):
        nc.sync.dma_start(out=slabD[:, oD3 + kt * 16:oD3 + kt * 16 + 10],
                          in_=t["whw"][kt * 128:(kt + 1) * 128, :])

    # head1
    hh1 = []
    for mt in range(8):
        ph1 = ptr.tile([128, 1], F32, name="ph1", tag="ptr")
        for kt in range(16):
            nc.tensor.matmul(
                ph1, lhsT=slabD[:, kt * 1024 + mt * 128: kt * 1024 + (mt + 1) * 128],
                rhs=pooled[kt], start=(kt == 0), stop=(kt == 15))
        h1c = sb.tile([128, 1], F16, name=f"hh1_{mt}", tag=f"hh1_{mt}")
        nc.scalar.activation(out=h1c, in_=ph1, func=AF.Relu, bias=bh1c[mt])
        hh1.append(h1c)
    # head2
    hh2 = []
    for mt in range(4):
        ph2 = ptr.tile([128, 1], F32, name="ph2", tag="ptr")
        for kt in range(8):
            nc.tensor.matmul(
                ph2, lhsT=slabD[:, oD2 + kt * 512 + mt * 128: oD2 + kt * 512 + (mt + 1) * 128],
                rhs=hh1[kt], start=(kt == 0), stop=(kt == 7))
        h2c = sb.tile([128, 1], F16, name=f"hh2_{mt}", tag=f"hh2_{mt}")
        nc.scalar.activation(out=h2c, in_=ph2, func=AF.Relu, bias=bh2c[mt])
        hh2.append(h2c)
    # final linear
    po = ptr.tile([10, 1], F32, name="po", tag="ptr")
    for kt in range(4):
        nc.tensor.matmul(po, lhsT=slabD[:, oD3 + kt * 16:oD3 + kt * 16 + 10],
                         rhs=hh2[kt], start=(kt == 0), stop=(kt == 3))
    osb = sb.tile([10, 1], F32, name="osb", tag="osb")
    nc.vector.tensor_scalar(osb, po, bwc[0][:10], None, op0=OP.add)
    nc.sync.dma_start(out=t["out"], in_=osb)


def _build_program():
    if "prog" in _PROG_CACHE:
        return _PROG_CACHE["prog"]
    from contextlib import ExitStack
    nc = bacc.Bacc("TRN2", target_bir_lowering=False, debug=False,
                   num_devices=N_CORES)
    t = _declare(nc)
    with tile.TileContext(nc) as tc:
        with ExitStack() as ctx:
            _emit(ctx, tc, t)
    nc.compile()
    _PROG_CACHE["prog"] = nc
    return nc


# --------------------------------------------------------------------------
# host side
# --------------------------------------------------------------------------

def _fold_params(params):
    def f32(a):
        return np.asarray(a, np.float32)

    def f64(a):
        return np.asarray(a, np.float64)

    (W1, b1, g1, bt1), = [tuple(map(f32, l)) for l in params["conv1"]]
    conv2 = [tuple(map(f32, l)) for l in params["conv2"]]
    (W21, b21, g21, bt21), (W22, b22, g22, bt22), (W23, b23, g23, bt23) = conv2
    (Wl, bl, gl, btl), = [tuple(map(f32, l)) for l in params["lin1"]]
    (Wh1, bh1, gh1, bth1), = [tuple(map(f32, l)) for l in params["head1"]]
    (Wh2, bh2, gh2, bth2), = [tuple(map(f32, l)) for l in params["head2"]]
    WhW = f32(params["headW"])
    bhW = f32(params["headb"])

    ok = min(g1.min(), g23.min(), gl.min()) > 0
    d = {
        "wc1": W1[:1024] - W1[1024:], "wb1": W1[1024:],
        "b1v": b1, "g1v": g1, "bt1v": bt1,
        "wc2": W21[:512] - W21[512:], "wb2": W21[512:], "b21v": b21,
        "w22": g21[:, None] * W22,
        "b22v": (f64(bt21) @ f64(W22) + b22).astype(np.float32),
        "w23": g22[:, None] * W23,
        "b23v": (f64(bt22) @ f64(W23) + b23).astype(np.float32),
        "wlt": Wl[:512], "wlb": g23[:, None] * Wl[512:],
        "blv": (f64(bl) + f64(bt23) @ f64(Wl[512:])).astype(np.float32),
        "wh1": gl[:, None] * Wh1,
        "bh1v": (f64(btl) @ f64(Wh1) + bh1).astype(np.float32),
        "wh2": gh1[:, None] * Wh2,
        "bh2v": (f64(bth1) @ f64(Wh2) + bh2).astype(np.float32),
        "whw": gh2[:, None] * WhW,
        "bwv": (f64(bth2) @ f64(WhW) + bhW).astype(np.float32),
    }
    casts = {"wc1", "wb1", "wc2", "wb2", "w22", "w23", "wlt", "wlb",
             "wh1", "wh2", "whw"}
    out = {}
    for k, v in d.items():
        out[k] = np.ascontiguousarray(
            v.astype(np.float16) if k in casts else v.astype(np.float32))
    out["jidx"] = np.arange(512, dtype=np.float32)
    out["ident"] = np.eye(128, dtype=np.float32)
    return out, ok


def _reference_numpy(x, params):
    """Exact eval-mode fallback (never used for the shipped seed, where all
    fold-relevant BN scales are positive)."""
    x = np.asarray(x, np.float32)
    B = 8
    Pn = x.shape[0] // B
    xg = x.reshape(B, Pn, -1)

    def mlp(a, layers):
        for (W, b, g, bt) in layers:
            a = np.maximum(a @ np.asarray(W, np.float32) + np.asarray(b, np.float32), 0)
            a = a * np.asarray(g, np.float32) + np.asarray(bt, np.float32)
        return a

    def edge_conv(a, layers):
        x2 = (a * a).sum(-1)
        d2 = x2[:, :, None] + x2[:, None, :] - 2.0 * np.einsum("bic,bjc->bij", a, a)
        idx = np.argsort(d2, axis=2, kind="stable")[:, :, :K]
        out = []
        for bi in range(B):
            xj = a[bi][idx[bi]]
            xi = np.broadcast_to(a[bi][:, None, :], xj.shape)
            feat = np.concatenate([xi, xj - xi], -1)
            out.append(mlp(feat, layers).max(1))
        return np.stack(out)

    x1 = edge_conv(xg, params["conv1"])
    x2_ = edge_conv(x1, params["conv2"])
    o = mlp(np.concatenate([x1, x2_], -1), params["lin1"]).max(1)
    o = mlp(o, params["head1"])
    o = mlp(o, params["head2"])
    return (o @ np.asarray(params["headW"], np.float32)
            + np.asarray(params["headb"], np.float32)).astype(np.float32)


def kernel(x, batch=None, num_graphs=None, params=None, **_):
    x = np.ascontiguousarray(np.asarray(x, np.float32))
    folded, ok = _fold_params(params)
    if not ok:
        return _reference_numpy(x, params)

    nc = _build_program()
    in_maps = []
    for c in range(N_CORES):
        m = dict(folded)
        m["xg"] = x[c * P:(c + 1) * P]
        in_maps.append(m)
    res = run_bass_kernel_spmd(nc, in_maps, list(range(N_CORES)))
    return np.stack([res.results[c]["out"] for c in range(N_CORES)]).astype(np.float32)


if __name__ == "__main__":
    _build_program()
    print("program built + compiled OK")


# revision 6
# speedup vs baseline: 12.5115x; 12.5115x over previous
"""Trainium2 Bass kernel for nn_DynamicModel_79955111182517 (DGCNN-style
EdgeConv GNN, B=8 graphs x P=512 points x C=1024, k=16 kNN).

Strategy (data-parallel, one graph per NeuronCore):
- BatchNorm (eval) scale/shift folded into adjacent matmuls on the host;
  max-aggregations commute with the positive per-channel scales.
- EdgeConv message nn([x_i, x_j - x_i]) is decomposed: feat @ W =
  x_i @ (Wtop - Wbot) + x_j @ Wbot, so only two P x C matmuls are needed
  instead of P*k x 2C.
- conv1 (single-layer MLP) factorizes fully: max_k relu(c_i + b_j) =
  relu(c_i + max_{j in knn(i)} b_j).
- kNN: fp32 Gram matrix on the PE, top-16 per row via the DVE max8 /
  max_index / match_replace ISA ops; neighbor gathers are one-hot fp16
  matmuls on the PE (keeps everything feature-major for the MLP chain).
- All heavy matmuls run in fp16 (full PE rate, ~8x finer rounding than
  bf16); distances in fp32 so neighbor selection matches the reference.
"""

import sys
import numpy as np

try:
    import concourse.bass as bass  # noqa: F401
except ImportError:
    sys.path.insert(0, "/opt/trn_rl_repo")

import concourse.bass as bass
import concourse.tile as tile
from concourse import bacc, mybir
from concourse.bass_utils import run_bass_kernel_spmd

F32 = mybir.dt.float32
F16 = mybir.dt.float16
U32 = mybir.dt.uint32
AF = mybir.ActivationFunctionType
OP = mybir.AluOpType

N_CORES = 8
P = 512          # points per graph
C0 = 1024        # input channels
K = 16           # neighbors
NI = P // 128    # 4 token tiles
CH = 512         # gather-chunk columns (32 points x 16 ranks)
NCH = (P * K) // CH  # 16 chunks
NEG = -1.0e30

_PROG_CACHE = {}


# --------------------------------------------------------------------------
# device program
# --------------------------------------------------------------------------

def _declare(nc):
    t = {}

    def inp(name, shape, dt):
        t[name] = nc.dram_tensor(name, list(shape), dt, kind="ExternalInput").ap()

    inp("xg", [P, C0], F32)
    inp("wc1", [1024, 512], F16)
    inp("wb1", [1024, 512], F16)
    inp("b1v", [512], F32)
    inp("g1v", [512], F32)
    inp("bt1v", [512], F32)
    inp("wc2", [512, 512], F16)
    inp("wb2", [512, 512], F16)
    inp("b21v", [512], F32)
    inp("w22", [512, 1024], F16)
    inp("b22v", [1024], F32)
    inp("w23", [1024, 1024], F16)
    inp("b23v", [1024], F32)
    inp("wlt", [512, 2048], F16)
    inp("wlb", [1024, 2048], F16)
    inp("blv", [2048], F32)
    inp("wh1", [2048, 1024], F16)
    inp("bh1v", [1024], F32)
    inp("wh2", [1024, 512], F16)
    inp("bh2v", [512], F32)
    inp("whw", [512, 10], F16)
    inp("bwv", [10], F32)
    inp("jidx", [512], F32)         # arange(512) for one-hot compares
    inp("ident", [128, 128], F32)   # PE-transpose identity

    t["out"] = nc.dram_tensor("out", [10], F32, kind="ExternalOutput").ap()
    # internal DRAM bounce buffers (partition -> row reshuffles)
    t["irow1"] = nc.dram_tensor("irow1", [P * K], F16).ap()
    t["irow2"] = nc.dram_tensor("irow2", [P * K], F16).ap()
    t["xrow1"] = nc.dram_tensor("xrow1", [P], F32).ap()
    t["xrow2"] = nc.dram_tensor("xrow2", [P], F32).ap()
    return t


def _make_pools(ctx, tc):
    return dict(
        wa=ctx.enter_context(tc.tile_pool(name="wa", bufs=1)),
        wb=ctx.enter_context(tc.tile_pool(name="wb", bufs=1)),
        sb=ctx.enter_context(tc.tile_pool(name="sb", bufs=1)),
        cols=ctx.enter_context(tc.tile_pool(name="cols", bufs=1)),
        pmm=ctx.enter_context(tc.tile_pool(name="pmm", bufs=6, space="PSUM")),
        ptr=ctx.enter_context(tc.tile_pool(name="ptr", bufs=2, space="PSUM")),
    )


def _emit(ctx, tc, t, pools=None):
    nc = tc.nc
    if pools is None:
        pools = _make_pools(ctx, tc)
    wa, wb, sb, cols, pmm, ptr = (pools[k] for k in
                                  ("wa", "wb", "sb", "cols", "pmm", "ptr"))

    def col_load(vec_ap, n128, base_name, dt=F32):
        out = []
        for i in range(n128):
            n = min(128, vec_ap.shape[0] - i * 128)
            c = cols.tile([n, 1], dt, name=f"{base_name}{i}", tag=f"{base_name}{i}")
            nc.sync.dma_start(out=c, in_=vec_ap[i * 128:i * 128 + n])
            out.append(c)
        return out

    # ---- constant columns ----
    b1c = col_load(t["b1v"], 4, "b1c")
    g1c = col_load(t["g1v"], 4, "g1c")
    bt1c = col_load(t["bt1v"], 4, "bt1c")
    b21c = col_load(t["b21v"], 4, "b21c")
    b22c = col_load(t["b22v"], 8, "b22c")
    b23c = col_load(t["b23v"], 8, "b23c")
    blc = col_load(t["blv"], 16, "blc")
    bh1c = col_load(t["bh1v"], 8, "bh1c")
    bh2c = col_load(t["bh2v"], 4, "bh2c")
    bwc = col_load(t["bwv"], 1, "bwc")
    jc = col_load(t["jidx"], 4, "jc", dt=F32)
    identw = cols.tile([128, 128], F32, name="identw", tag="identw")
    nc.sync.dma_start(out=identw, in_=t["ident"])

    # ---- weight slab A: conv1 (later reused for lin1) ----
    # layout: [wc1 8kt x 512 | wb1 8kt x 512]
    slabA = wa.tile([128, 16 * 512], F16, name="slabA", tag="wslabA")
    for kt in range(8):
        nc.sync.dma_start(out=slabA[:, kt * 512:(kt + 1) * 512],
                          in_=t["wc1"][kt * 128:(kt + 1) * 128, :])
        nc.sync.dma_start(out=slabA[:, (8 + kt) * 512:(9 + kt) * 512],
                          in_=t["wb1"][kt * 128:(kt + 1) * 128, :])

    def wA1(kt, mt):  # wc1 lhsT block [128k, 128m]
        return slabA[:, kt * 512 + mt * 128: kt * 512 + (mt + 1) * 128]

    def wA2(kt):      # wb1 rhs k-tile [128k, 512]
        return slabA[:, (8 + kt) * 512:(9 + kt) * 512]

    # ---- weight slab B: conv2 (later reused for heads) ----
    # layout: [wc2 4x512 | wb2 4x512 | w22 4x1024 | w23 8x1024]
    slabB = wb.tile([128, 2048 + 2048 + 4096 + 8192], F16, name="slabB", tag="wslabB")
    oB2 = 2048
    oB3 = 4096
    oB4 = 8192
    for kt in range(4):
        nc.sync.dma_start(out=slabB[:, kt * 512:(kt + 1) * 512],
                          in_=t["wc2"][kt * 128:(kt + 1) * 128, :])
        nc.sync.dma_start(out=slabB[:, oB2 + kt * 512:oB2 + (kt + 1) * 512],
                          in_=t["wb2"][kt * 128:(kt + 1) * 128, :])
        nc.sync.dma_start(out=slabB[:, oB3 + kt * 1024:oB3 + (kt + 1) * 1024],
                          in_=t["w22"][kt * 128:(kt + 1) * 128, :])
    for kt in range(8):
        nc.sync.dma_start(out=slabB[:, oB4 + kt * 1024:oB4 + (kt + 1) * 1024],
                          in_=t["w23"][kt * 128:(kt + 1) * 128, :])

    def wB_c2(kt, mt):
        return slabB[:, kt * 512 + mt * 128: kt * 512 + (mt + 1) * 128]

    def wB_b2(kt):
        return slabB[:, oB2 + kt * 512:oB2 + (kt + 1) * 512]

    def wB_22(kt, nt):
        return slabB[:, oB3 + kt * 1024 + nt * 128: oB3 + kt * 1024 + (nt + 1) * 128]

    def wB_23(kt, nt):
        return slabB[:, oB4 + kt * 1024 + nt * 128: oB4 + kt * 1024 + (nt + 1) * 128]

    # ==================================================================
    # stage 0/1: load X blockwise, transpose on PE -> XT (fp32 + fp16)
    # ==================================================================
    xt32 = [sb.tile([128, P], F32, name=f"xt32_{ct}", tag=f"xt32_{ct}") for ct in range(8)]
    xt16 = [sb.tile([128, P], F16, name=f"xt16_{ct}", tag=f"xt16_{ct}") for ct in range(8)]
    for ct in range(8):
        for it in range(NI):
            xb = sb.tile([128, 128], F32, name="xb", tag="xblk", bufs=6)
            nc.sync.dma_start(
                out=xb, in_=t["xg"][it * 128:(it + 1) * 128, ct * 128:(ct + 1) * 128])
            pt = ptr.tile([128, 128], F32, name="pt", tag="ptr")
            nc.tensor.transpose(pt, xb, identw)
            nc.scalar.activation(out=xt32[ct][:, it * 128:(it + 1) * 128], in_=pt, func=AF.Copy)
        nc.vector.tensor_copy(out=xt16[ct], in_=xt32[ct])

    # ==================================================================
    # generic helpers
    # ==================================================================
    def gram_topk_idxrow(xt_f32, nk, gsb_tag, irow_dram, xrow_dram, conv_id):
        """Gram matrix (fp32), scores s = 2G - x2_j, top-16 indices per row,
        assembled into a broadcast index row [128, P*K] (f16 values)."""
        gsb = []
        for mt in range(NI):
            pg = pmm.tile([128, P], F32, name="pg", tag="pmm")
            for kt in range(nk):
                nc.tensor.matmul(pg, lhsT=xt_f32[kt][:, mt * 128:(mt + 1) * 128],
                                 rhs=xt_f32[kt], start=(kt == 0), stop=(kt == nk - 1))
            g = sb.tile([128, P], F32, name=f"g{conv_id}_{mt}", tag=f"{gsb_tag}{mt}")
            nc.scalar.activation(out=g, in_=pg, func=AF.Copy)
            gsb.append(g)

        # x2 row from the diagonal of G
        for mt in range(NI):
            dsel = sb.tile([128, P], F32, name="dsel", tag="dsel", bufs=2)
            nc.gpsimd.affine_select(
                out=dsel, in_=gsb[mt], compare_op=OP.is_equal, fill=0.0,
                base=128 * mt, pattern=[[-1, P]], channel_multiplier=1)
            dcol = sb.tile([128, 1], F32, name="dcol", tag="dcol", bufs=2)
            nc.vector.reduce_sum(out=dcol, in_=dsel, axis=mybir.AxisListType.X)
            nc.sync.dma_start(out=xrow_dram[mt * 128:(mt + 1) * 128], in_=dcol)
        x2b = sb.tile([128, P], F32, name=f"x2b{conv_id}", tag="x2b")
        nc.sync.dma_start(out=x2b, in_=xrow_dram.rearrange("(o n) -> o n", o=1).to_broadcast([128, P]))

        # s = 2G - x2_j (in place over gsb); then top-16 via max8/match_replace
        for mt in range(NI):
            nc.vector.scalar_tensor_tensor(
                out=gsb[mt], in0=gsb[mt], scalar=2.0, in1=x2b,
                op0=OP.mult, op1=OP.subtract)
            va = sb.tile([128, 8], F32, name="va", tag="va", bufs=2)
            vb = sb.tile([128, 8], F32, name="vb", tag="vb", bufs=2)
            idxu = sb.tile([128, K], U32, name="idxu", tag="idxu", bufs=2)
            idxf = sb.tile([128, K], F16, name="idxf", tag="idxf", bufs=2)
            nc.vector.max(out=va, in_=gsb[mt])
            nc.vector.max_index(out=idxu[:, 0:8], in_max=va, in_values=gsb[mt])
            nc.vector.match_replace(out=gsb[mt], in_to_replace=va,
                                    in_values=gsb[mt], imm_value=NEG)
            nc.vector.max(out=vb, in_=gsb[mt])
            nc.vector.max_index(out=idxu[:, 8:16], in_max=vb, in_values=gsb[mt])
            nc.vector.tensor_copy(out=idxf, in_=idxu)
            nc.sync.dma_start(out=irow_dram[mt * 128 * K:(mt + 1) * 128 * K], in_=idxf)

        irow_b = sb.tile([128, P * K], F16, name=f"irow_b{conv_id}", tag="irow_b")
        nc.sync.dma_start(out=irow_b, in_=irow_dram.rearrange("(o n) -> o n", o=1).to_broadcast([128, P * K]))
        return irow_b

    def onehot_chunk(irow_b, ch, conv_id):
        ohs = []
        for jt in range(NI):
            oh = sb.tile([128, CH], F16, name="oh", tag=f"oh{jt}", bufs=2)
            nc.vector.tensor_scalar(
                oh, irow_b[:, ch * CH:(ch + 1) * CH], jc[jt], None, op0=OP.is_equal)
            ohs.append(oh)
        return ohs

    # ==================================================================
    # conv1
    # ==================================================================
    irow_b1 = gram_topk_idxrow(xt32, 8, "gsb", t["irow1"], t["xrow1"], 1)

    # c1 = X @ (Wtop-Wbot) + b1  (feature-major), b1tok = X @ Wbot (token-major)
    c1T = []
    for mt in range(NI):
        pc = pmm.tile([128, P], F32, name="pc", tag="pmm")
        for kt in range(8):
            nc.tensor.matmul(pc, lhsT=wA1(kt, mt), rhs=xt16[kt],
                             start=(kt == 0), stop=(kt == 7))
        c = sb.tile([128, P], F32, name=f"c1T{mt}", tag=f"cT{mt}")
        nc.vector.tensor_scalar(c, pc, b1c[mt], None, op0=OP.add)
        c1T.append(c)
    btok1 = []
    for jt in range(NI):
        pb = pmm.tile([128, 512], F32, name="pb", tag="pmm")
        for kt in range(8):
            nc.tensor.matmul(pb, lhsT=xt16[kt][:, jt * 128:(jt + 1) * 128],
                             rhs=wA2(kt), start=(kt == 0), stop=(kt == 7))
        bt_ = sb.tile([128, 512], F16, name=f"btok1{jt}", tag=f"btok{jt}")
        nc.vector.tensor_copy(out=bt_, in_=pb)
        btok1.append(bt_)

    # gather chunks: T[c, i] = max_r b[knn_r(i), c]
    Tt = [sb.tile([128, P], F32, name=f"Tt{ct}", tag=f"xt32_{ct}") for ct in range(NI)]
    for ch in range(NCH):
        ohs = onehot_chunk(irow_b1, ch, 1)
        for ct in range(NI):
            pgt = pmm.tile([128, CH], F32, name="pgt", tag="pmm")
            for jt in range(NI):
                nc.tensor.matmul(pgt, lhsT=btok1[jt][:, ct * 128:(ct + 1) * 128],
                                 rhs=ohs[jt], start=(jt == 0), stop=(jt == NI - 1))
            nc.vector.tensor_reduce(
                out=Tt[ct][:, ch * (CH // K):(ch + 1) * (CH // K)],
                in_=pgt.rearrange("p (i r) -> p i r", r=K),
                axis=mybir.AxisListType.X, op=OP.max)

    # x1 = g1 * relu(c1 + T) + bt1   (keep fp32 for Gram, fp16 for matmuls)
    x1_32 = []
    x1_16 = []
    for ct in range(NI):
        nc.vector.tensor_add(c1T[ct], c1T[ct], Tt[ct])
        x32 = sb.tile([128, P], F32, name=f"x1_32_{ct}", tag=f"xt32_{4 + ct}")
        nc.scalar.activation(out=x32, in_=c1T[ct], func=AF.Relu)
        nc.vector.tensor_scalar(x32, x32, g1c[ct], bt1c[ct], op0=OP.mult, op1=OP.add)
        x16 = sb.tile([128, P], F16, name=f"x1_16_{ct}", tag=f"x1_16_{ct}")
        nc.vector.tensor_copy(out=x16, in_=x32)
        x1_32.append(x32)
        x1_16.append(x16)

    # ==================================================================
    # conv2
    # ==================================================================
    irow_b2 = gram_topk_idxrow(x1_32, NI, "gsb", t["irow2"], t["xrow2"], 2)

    c2T = []
    for mt in range(NI):
        pc2 = pmm.tile([128, P], F32, name="pc2", tag="pmm")
        for kt in range(NI):
            nc.tensor.matmul(pc2, lhsT=wB_c2(kt, mt), rhs=x1_16[kt],
                             start=(kt == 0), stop=(kt == NI - 1))
        c2 = sb.tile([128, P], F32, name=f"c2T{mt}", tag=f"cT{mt}")
        nc.vector.tensor_scalar(c2, pc2, b21c[mt], None, op0=OP.add)
        c2T.append(c2)
    btok2 = []
    for jt in range(NI):
        pb2 = pmm.tile([128, 512], F32, name="pb2", tag="pmm")
        for kt in range(NI):
            nc.tensor.matmul(pb2, lhsT=x1_16[kt][:, jt * 128:(jt + 1) * 128],
                             rhs=wB_b2(kt), start=(kt == 0), stop=(kt == NI - 1))
        bt2 = sb.tile([128, 512], F16, name=f"btok2{jt}", tag=f"btok{jt}")
        nc.vector.tensor_copy(out=bt2, in_=pb2)
        btok2.append(bt2)

    x2acc = [sb.tile([128, P], F32, name=f"x2acc{nt}", tag=f"xt32_{nt}") for nt in range(8)]
    IC = CH // K  # 32 points per chunk
    for ch in range(NCH):
        ohs = onehot_chunk(irow_b2, ch, 2)
        # gather + add c2 (broadcast over ranks) + relu -> h1 (fp16)
        h1 = []
        for ct in range(NI):
            pg2 = pmm.tile([128, CH], F32, name="pg2", tag="pmm")
            for jt in range(NI):
                nc.tensor.matmul(pg2, lhsT=btok2[jt][:, ct * 128:(ct + 1) * 128],
                                 rhs=ohs[jt], start=(jt == 0), stop=(jt == NI - 1))
            h = sb.tile([128, CH], F16, name="h1", tag=f"h1_{ct}", bufs=2)
            cbc = c2T[ct][:, ch * IC:(ch + 1) * IC].rearrange(
                "p (i o) -> p i o", o=1).to_broadcast([128, IC, K])
            nc.vector.tensor_tensor(
                out=h.rearrange("p (i r) -> p i r", r=K),
                in0=pg2.rearrange("p (i r) -> p i r", r=K),
                in1=cbc, op=OP.add)
            nc.scalar.activation(out=h, in_=h, func=AF.Relu)
            h1.append(h)
        # layer2: h2 = relu(h1 @ W22f + b22f)
        h2 = []
        for nt in range(8):
            pz2 = pmm.tile([128, CH], F32, name="pz2", tag="pmm")
            for kt in range(NI):
                nc.tensor.matmul(pz2, lhsT=wB_22(kt, nt), rhs=h1[kt],
                                 start=(kt == 0), stop=(kt == NI - 1))
            hh = sb.tile([128, CH], F16, name="h2", tag=f"h2_{nt}", bufs=2)
            nc.scalar.activation(out=hh, in_=pz2, func=AF.Relu, bias=b22c[nt])
            h2.append(hh)
        # layer3 + max over ranks
        for nt in range(8):
            pz3 = pmm.tile([128, CH], F32, name="pz3", tag="pmm")
            for kt in range(8):
                nc.tensor.matmul(pz3, lhsT=wB_23(kt, nt), rhs=h2[kt],
                                 start=(kt == 0), stop=(kt == 7))
            nc.vector.tensor_reduce(
                out=x2acc[nt][:, ch * IC:(ch + 1) * IC],
                in_=pz3.rearrange("p (i r) -> p i r", r=K),
                axis=mybir.AxisListType.X, op=OP.max)

    # x2relu = relu(x2acc + b23f) -> fp16
    x2r = []
    for nt in range(8):
        xx = sb.tile([128, P], F16, name=f"x2r{nt}", tag=f"x2r{nt}")
        nc.scalar.activation(out=xx, in_=x2acc[nt], func=AF.Relu, bias=b23c[nt])
        x2r.append(xx)

    # ---- weight slab C: lin1 (reuses slab A space) ----
    slabC = wa.tile([128, 12 * 2048], F16, name="slabC", tag="wslabA")
    for kt in range(4):
        nc.sync.dma_start(out=slabC[:, kt * 2048:(kt + 1) * 2048],
                          in_=t["wlt"][kt * 128:(kt + 1) * 128, :])
    for kt in range(8):
        nc.sync.dma_start(out=slabC[:, (4 + kt) * 2048:(5 + kt) * 2048],
                          in_=t["wlb"][kt * 128:(kt + 1) * 128, :])

    # lin1 + global max pool -> pooled [2048] (fp16 cols)
    pooled = []
    for mt in range(16):
        pzl = pmm.tile([128, P], F32, name="pzl", tag="pmm")
        for kt in range(4):
            nc.tensor.matmul(
                pzl, lhsT=slabC[:, kt * 2048 + mt * 128: kt * 2048 + (mt + 1) * 128],
                rhs=x1_16[kt], start=(kt == 0), stop=False)
        for kt in range(8):
            nc.tensor.matmul(
                pzl, lhsT=slabC[:, (4 + kt) * 2048 + mt * 128: (4 + kt) * 2048 + (mt + 1) * 128],
                rhs=x2r[kt], start=False, stop=(kt == 7))
        pcol = sb.tile([128, 1], F32, name="pcol", tag="pcol", bufs=4)
        nc.vector.tensor_reduce(out=pcol, in_=pzl, axis=mybir.AxisListType.X, op=OP.max)
        pc16 = sb.tile([128, 1], F16, name=f"pool{mt}", tag=f"pool{mt}")
        nc.scalar.activation(out=pc16, in_=pcol, func=AF.Relu, bias=blc[mt])
        pooled.append(pc16)

    # ---- weight slab D: heads (reuses slab B space) ----
    slabD = wb.tile([128, 16 * 1024 + 8 * 512 + 4 * 16], F16, name="slabD", tag="wslabB")
    oD2 = 16 * 1024
    oD3 = oD2 + 8 * 512
    for kt in range(16):
        nc.sync.dma_start(out=slabD[:, kt * 1024:(kt + 1) * 1024],
                          in_=t["wh1"][kt * 128:(kt + 1) * 128, :])
    for kt in range(8):
        nc.sync.dma_start(out=slabD[:, oD2 + kt * 512:oD2 + (kt + 1) * 512],
                          in_=t["wh2"][kt * 128:(kt + 1) * 128, :])
    for kt in range(4):
        nc.sync.dma_start(out=slabD[:, oD3 + kt * 16:oD3 + kt * 16 + 10],
                          in_=t["whw"][kt * 128:(kt + 1) * 128, :])

    # head1
    hh1 = []
    for mt in range(8):
        ph1 = ptr.tile([128, 1], F32, name="ph1", tag="ptr")
        for kt in range(16):
            nc.tensor.matmul(
                ph1, lhsT=slabD[:, kt * 1024 + mt * 128: kt * 1024 + (mt + 1) * 128],
                rhs=pooled[kt], start=(kt == 0), stop=(kt == 15))
        h1c = sb.tile([128, 1], F16, name=f"hh1_{mt}", tag=f"hh1_{mt}")
        nc.scalar.activation(out=h1c, in_=ph1, func=AF.Relu, bias=bh1c[mt])
        hh1.append(h1c)
    # head2
    hh2 = []
    for mt in range(4):
        ph2 = ptr.tile([128, 1], F32, name="ph2", tag="ptr")
        for kt in range(8):
            nc.tensor.matmul(
                ph2, lhsT=slabD[:, oD2 + kt * 512 + mt * 128: oD2 + kt * 512 + (mt + 1) * 128],
                rhs=hh1[kt], start=(kt == 0), stop=(kt == 7))
        h2c = sb.tile([128, 1], F16, name=f"hh2_{mt}", tag=f"hh2_{mt}")
        nc.scalar.activation(out=h2c, in_=ph2, func=AF.Relu, bias=bh2c[mt])
        hh2.append(h2c)
    # final linear
    po = ptr.tile([10, 1], F32, name="po", tag="ptr")
    for kt in range(4):
        nc.tensor.matmul(po, lhsT=slabD[:, oD3 + kt * 16:oD3 + kt * 16 + 10],
                         rhs=hh2[kt], start=(kt == 0), stop=(kt == 3))
    osb = sb.tile([10, 1], F32, name="osb", tag="osb")
    nc.vector.tensor_scalar(osb, po, bwc[0][:10], None, op0=OP.add)
    nc.sync.dma_start(out=t["out"], in_=osb)


def _build_program():
    if "prog" in _PROG_CACHE:
        return _PROG_CACHE["prog"]
    from contextlib import ExitStack
    nc = bacc.Bacc("TRN2", target_bir_lowering=False, debug=False,
                   num_devices=N_CORES)
    t = _declare(nc)
    with tile.TileContext(nc) as tc:
        with ExitStack() as ctx:
            _emit(ctx, tc, t)
    nc.compile()
    _PROG_CACHE["prog"] = nc
    return nc


# --------------------------------------------------------------------------
# host side
# --------------------------------------------------------------------------

def _fold_params(params):
    def f32(a):
        return np.asarray(a, np.float32)

    def f64(a):
        return np.asarray(a, np.float64)

    (W1, b1, g1, bt1), = [tuple(map(f32, l)) for l in params["conv1"]]
    conv2 = [tuple(map(f32, l)) for l in params["conv2"]]
    (W21, b21, g21, bt21), (W22, b22, g22, bt22), (W23, b23, g23, bt23) = conv2
    (Wl, bl, gl, btl), = [tuple(map(f32, l)) for l in params["lin1"]]
    (Wh1, bh1, gh1, bth1), = [tuple(map(f32, l)) for l in params["head1"]]
    (Wh2, bh2, gh2, bth2), = [tuple(map(f32, l)) for l in params["head2"]]
    WhW = f32(params["headW"])
    bhW = f32(params["headb"])

    ok = min(g1.min(), g23.min(), gl.min()) > 0
    d = {
        "wc1": W1[:1024] - W1[1024:], "wb1": W1[1024:],
        "b1v": b1, "g1v": g1, "bt1v": bt1,
        "wc2": W21[:512] - W21[512:], "wb2": W21[512:], "b21v": b21,
        "w22": g21[:, None] * W22,
        "b22v": (f64(bt21) @ f64(W22) + b22).astype(np.float32),
        "w23": g22[:, None] * W23,
        "b23v": (f64(bt22) @ f64(W23) + b23).astype(np.float32),
        "wlt": Wl[:512], "wlb": g23[:, None] * Wl[512:],
        "blv": (f64(bl) + f64(bt23) @ f64(Wl[512:])).astype(np.float32),
        "wh1": gl[:, None] * Wh1,
        "bh1v": (f64(btl) @ f64(Wh1) + bh1).astype(np.float32),
        "wh2": gh1[:, None] * Wh2,
        "bh2v": (f64(bth1) @ f64(Wh2) + bh2).astype(np.float32),
        "whw": gh2[:, None] * WhW,
        "bwv": (f64(bth2) @ f64(WhW) + bhW).astype(np.float32),
    }
    casts = {"wc1", "wb1", "wc2", "wb2", "w22", "w23", "wlt", "wlb",
             "wh1", "wh2", "whw"}
    out = {}
    for k, v in d.items():
        out[k] = np.ascontiguousarray(
            v.astype(np.float16) if k in casts else v.astype(np.float32))
    out["jidx"] = np.arange(512, dtype=np.float32)
    out["ident"] = np.eye(128, dtype=np.float32)
    return out, ok


def _reference_numpy(x, params):
    """Exact eval-mode fallback (never used for the shipped seed, where all
    fold-relevant BN scales are positive)."""
    x = np.asarray(x, np.float32)
    B = 8
    Pn = x.shape[0] // B
    xg = x.reshape(B, Pn, -1)

    def mlp(a, layers):
        for (W, b, g, bt) in layers:
            a = np.maximum(a @ np.asarray(W, np.float32) + np.asarray(b, np.float32), 0)
            a = a * np.asarray(g, np.float32) + np.asarray(bt, np.float32)
        return a

    def edge_conv(a, layers):
        x2 = (a * a).sum(-1)
        d2 = x2[:, :, None] + x2[:, None, :] - 2.0 * np.einsum("bic,bjc->bij", a, a)
        idx = np.argsort(d2, axis=2, kind="stable")[:, :, :K]
        out = []
        for bi in range(B):
            xj = a[bi][idx[bi]]
            xi = np.broadcast_to(a[bi][:, None, :], xj.shape)
            feat = np.concatenate([xi, xj - xi], -1)
            out.append(mlp(feat, layers).max(1))
        return np.stack(out)

    x1 = edge_conv(xg, params["conv1"])
    x2_ = edge_conv(x1, params["conv2"])
    o = mlp(np.concatenate([x1, x2_], -1), params["lin1"]).max(1)
    o = mlp(o, params["head1"])
    o = mlp(o, params["head2"])
    return (o @ np.asarray(params["headW"], np.float32)
            + np.asarray(params["headb"], np.float32)).astype(np.float32)


def kernel(x, batch=None, num_graphs=None, params=None, **_):
    x = np.ascontiguousarray(np.asarray(x, np.float32))
    folded, ok = _fold_params(params)
    if not ok:
        return _reference_numpy(x, params)

    nc = _build_program()
    in_maps = []
    for c in range(N_CORES):
        m = dict(folded)
        m["xg"] = x[c * P:(c + 1) * P]
        in_maps.append(m)
    res = run_bass_kernel_spmd(nc, in_maps, list(range(N_CORES)))
    return np.stack([res.results[c]["out"] for c in range(N_CORES)]).astype(np.float32)


if __name__ == "__main__":
    _build_program()
    print("program built + compiled OK")


# revision 10
# speedup vs baseline: 15.5486x; 1.2428x over previous
"""Trainium2 Bass kernel for nn_DynamicModel_79955111182517 (DGCNN-style
EdgeConv GNN, B=8 graphs x P=512 points x C=1024, k=16 kNN).

Strategy (data-parallel, one graph per NeuronCore):
- BatchNorm (eval) scale/shift folded into adjacent matmuls on the host;
  max-aggregations commute with the positive per-channel scales.
- EdgeConv message nn([x_i, x_j - x_i]) is decomposed: feat @ W =
  x_i @ (Wtop - Wbot) + x_j @ Wbot, so only two P x C matmuls are needed
  instead of P*k x 2C.
- conv1 (single-layer MLP) factorizes fully: max_k relu(c_i + b_j) =
  relu(c_i + max_{j in knn(i)} b_j).
- kNN: fp32 Gram matrix on the PE, top-16 per row via the DVE max8 /
  max_index / match_replace ISA ops; neighbor gathers are one-hot fp16
  matmuls on the PE (keeps everything feature-major for the MLP chain).
- All heavy matmuls run in fp16 (full PE rate, ~8x finer rounding than
  bf16); distances in fp32 so neighbor selection matches the reference.
"""

import sys
import numpy as np

try:
    import concourse.bass as bass  # noqa: F401
except ImportError:
    sys.path.insert(0, "/opt/trn_rl_repo")

import concourse.bass as bass
import concourse.tile as tile
from concourse import bacc, mybir
from concourse.bass_utils import run_bass_kernel_spmd

F32 = mybir.dt.float32
F16 = mybir.dt.float16
U32 = mybir.dt.uint32
I16 = mybir.dt.int16
AF = mybir.ActivationFunctionType
OP = mybir.AluOpType

N_CORES = 8
P = 512          # points per graph
C0 = 1024        # input channels
K = 16           # neighbors
NI = P // 128    # 4 token tiles
CH = 512         # gather-chunk columns (32 points x 16 ranks)
NCH = (P * K) // CH  # 16 chunks
NEG = -1.0e30
USE_DMA_GATHER = True

_PROG_CACHE = {}


# --------------------------------------------------------------------------
# device program
# --------------------------------------------------------------------------

def _declare(nc):
    t = {}

    def inp(name, shape, dt):
        t[name] = nc.dram_tensor(name, list(shape), dt, kind="ExternalInput").ap()

    inp("xg", [P, C0], F32)
    inp("wc1", [1024, 512], F16)
    inp("wb1", [1024, 512], F16)
    inp("b1v", [512], F32)
    inp("g1v", [512], F32)
    inp("bt1v", [512], F32)
    inp("wc2", [512, 512], F16)
    inp("wb2", [512, 512], F16)
    inp("b21v", [512], F32)
    inp("w22", [512, 1024], F16)
    inp("b22v", [1024], F32)
    inp("w23", [1024, 1024], F16)
    inp("b23v", [1024], F32)
    inp("wlt", [512, 2048], F16)
    inp("wlb", [1024, 2048], F16)
    inp("blv", [2048], F32)
    inp("wh1", [2048, 1024], F16)
    inp("bh1v", [1024], F32)
    inp("wh2", [1024, 512], F16)
    inp("bh2v", [512], F32)
    inp("whw", [512, 10], F16)
    inp("bwv", [10], F32)
    inp("jidx", [512], F32)         # arange(512) for one-hot compares
    inp("ident", [128, 128], F32)   # PE-transpose identity
    inp("ident16", [128, 128], F16)

    t["out"] = nc.dram_tensor("out", [10], F32, kind="ExternalOutput").ap()
    # internal DRAM bounce buffers (partition -> row reshuffles)
    t["irow1"] = nc.dram_tensor("irow1", [P * K], F16).ap()
    t["irow2"] = nc.dram_tensor("irow2", [P * K], F16).ap()
    t["xrow1"] = nc.dram_tensor("xrow1", [P], F32).ap()
    t["xrow2"] = nc.dram_tensor("xrow2", [P], F32).ap()
    return t


def _make_pools(ctx, tc):
    return dict(
        wa=ctx.enter_context(tc.tile_pool(name="wa", bufs=1)),
        wb=ctx.enter_context(tc.tile_pool(name="wb", bufs=1)),
        sb=ctx.enter_context(tc.tile_pool(name="sb", bufs=1)),
        cols=ctx.enter_context(tc.tile_pool(name="cols", bufs=1)),
        pmm=ctx.enter_context(tc.tile_pool(name="pmm", bufs=6, space="PSUM")),
        ptr=ctx.enter_context(tc.tile_pool(name="ptr", bufs=2, space="PSUM")),
    )


def _emit(ctx, tc, t, pools=None):
    nc = tc.nc
    if pools is None:
        pools = _make_pools(ctx, tc)
    wa, wb, sb, cols, pmm, ptr = (pools[k] for k in
                                  ("wa", "wb", "sb", "cols", "pmm", "ptr"))

    def col_load(vec_ap, n128, base_name, dt=F32):
        out = []
        for i in range(n128):
            n = min(128, vec_ap.shape[0] - i * 128)
            c = cols.tile([n, 1], dt, name=f"{base_name}{i}", tag=f"{base_name}{i}")
            nc.sync.dma_start(out=c, in_=vec_ap[i * 128:i * 128 + n])
            out.append(c)
        return out

    # ---- constant columns ----
    b1c = col_load(t["b1v"], 4, "b1c")
    g1c = col_load(t["g1v"], 4, "g1c")
    bt1c = col_load(t["bt1v"], 4, "bt1c")
    b21c = col_load(t["b21v"], 4, "b21c")
    b22c = col_load(t["b22v"], 8, "b22c")
    b23c = col_load(t["b23v"], 8, "b23c")
    blc = col_load(t["blv"], 16, "blc")
    bh1c = col_load(t["bh1v"], 8, "bh1c")
    bh2c = col_load(t["bh2v"], 4, "bh2c")
    bwc = col_load(t["bwv"], 1, "bwc")
    jc = col_load(t["jidx"], 4, "jc", dt=F32)
    identw = cols.tile([128, 128], F32, name="identw", tag="identw")
    nc.sync.dma_start(out=identw, in_=t["ident"])
    identw16 = cols.tile([128, 128], F16, name="identw16", tag="identw16")
    nc.sync.dma_start(out=identw16, in_=t["ident16"])

    # ---- weight slab A: conv1 (later reused for lin1) ----
    # layout: [wc1 8kt x 512 | wb1 8kt x 512]
    slabA = wa.tile([128, 16 * 512], F16, name="slabA", tag="wslabA")
    for kt in range(8):
        nc.sync.dma_start(out=slabA[:, kt * 512:(kt + 1) * 512],
                          in_=t["wc1"][kt * 128:(kt + 1) * 128, :])
        nc.sync.dma_start(out=slabA[:, (8 + kt) * 512:(9 + kt) * 512],
                          in_=t["wb1"][kt * 128:(kt + 1) * 128, :])

    def wA1(kt, mt):  # wc1 lhsT block [128k, 128m]
        return slabA[:, kt * 512 + mt * 128: kt * 512 + (mt + 1) * 128]

    def wA2(kt):      # wb1 rhs k-tile [128k, 512]
        return slabA[:, (8 + kt) * 512:(9 + kt) * 512]

    # ---- weight slab B: conv2 (later reused for heads) ----
    # layout: [wc2 4x512 | wb2 4x512 | w22 4x1024 | w23 8x1024]
    slabB = wb.tile([128, 2048 + 2048 + 4096 + 8192], F16, name="slabB", tag="wslabB")
    oB2 = 2048
    oB3 = 4096
    oB4 = 8192
    for kt in range(4):
        nc.sync.dma_start(out=slabB[:, kt * 512:(kt + 1) * 512],
                          in_=t["wc2"][kt * 128:(kt + 1) * 128, :])
        nc.sync.dma_start(out=slabB[:, oB2 + kt * 512:oB2 + (kt + 1) * 512],
                          in_=t["wb2"][kt * 128:(kt + 1) * 128, :])
        nc.sync.dma_start(out=slabB[:, oB3 + kt * 1024:oB3 + (kt + 1) * 1024],
                          in_=t["w22"][kt * 128:(kt + 1) * 128, :])
    for kt in range(8):
        nc.sync.dma_start(out=slabB[:, oB4 + kt * 1024:oB4 + (kt + 1) * 1024],
                          in_=t["w23"][kt * 128:(kt + 1) * 128, :])

    def wB_c2(kt, mt):
        return slabB[:, kt * 512 + mt * 128: kt * 512 + (mt + 1) * 128]

    def wB_b2(kt):
        return slabB[:, oB2 + kt * 512:oB2 + (kt + 1) * 512]

    def wB_22(kt, nt):
        return slabB[:, oB3 + kt * 1024 + nt * 128: oB3 + kt * 1024 + (nt + 1) * 128]

    def wB_23(kt, nt):
        return slabB[:, oB4 + kt * 1024 + nt * 128: oB4 + kt * 1024 + (nt + 1) * 128]

    # ==================================================================
    # stage 0/1: load X blockwise, transpose on PE -> XT (fp32 + fp16)
    # ==================================================================
    xt32 = [sb.tile([128, P], F32, name=f"xt32_{ct}", tag=f"xt32_{ct}") for ct in range(8)]
    xt16 = [sb.tile([128, P], F16, name=f"xt16_{ct}", tag=f"xt16_{ct}") for ct in range(8)]
    for ct in range(8):
        for it in range(NI):
            xb = sb.tile([128, 128], F32, name="xb", tag="xblk", bufs=6)
            nc.sync.dma_start(
                out=xb, in_=t["xg"][it * 128:(it + 1) * 128, ct * 128:(ct + 1) * 128])
            pt = ptr.tile([128, 128], F32, name="pt", tag="ptr")
            nc.tensor.transpose(pt, xb, identw)
            nc.scalar.activation(out=xt32[ct][:, it * 128:(it + 1) * 128], in_=pt, func=AF.Copy)
        nc.vector.tensor_copy(out=xt16[ct], in_=xt32[ct])

    # ==================================================================
    # generic helpers
    # ==================================================================
    def gram_topk_idxrow(xt_f32, nk, gsb_tag, irow_dram, xrow_dram, conv_id):
        """Gram matrix (fp32), scores s = 2G - x2_j, top-16 indices per row,
        assembled into a broadcast index row [128, P*K] (f16 values)."""
        gsb = []
        for mt in range(NI):
            pg = pmm.tile([128, P], F32, name="pg", tag="pmm")
            for kt in range(nk):
                nc.tensor.matmul(pg, lhsT=xt_f32[kt][:, mt * 128:(mt + 1) * 128],
                                 rhs=xt_f32[kt], start=(kt == 0), stop=(kt == nk - 1))
            g = sb.tile([128, P], F32, name=f"g{conv_id}_{mt}", tag=f"{gsb_tag}{mt}")
            nc.scalar.activation(out=g, in_=pg, func=AF.Copy)
            gsb.append(g)

        # x2 row from the diagonal of G
        for mt in range(NI):
            dsel = sb.tile([128, P], F32, name="dsel", tag="dsel", bufs=2)
            nc.gpsimd.affine_select(
                out=dsel, in_=gsb[mt], compare_op=OP.is_equal, fill=0.0,
                base=128 * mt, pattern=[[-1, P]], channel_multiplier=1)
            dcol = sb.tile([128, 1], F32, name="dcol", tag="dcol", bufs=2)
            nc.vector.reduce_sum(out=dcol, in_=dsel, axis=mybir.AxisListType.X)
            nc.sync.dma_start(out=xrow_dram[mt * 128:(mt + 1) * 128], in_=dcol)
        x2b = sb.tile([128, P], F32, name=f"x2b{conv_id}", tag="x2b")
        nc.sync.dma_start(out=x2b, in_=xrow_dram.rearrange("(o n) -> o n", o=1).to_broadcast([128, P]))

        idxg = None
        if USE_DMA_GATHER:
            idxg = sb.tile([128, P], I16, name=f"idxg{conv_id}", tag="idxg", bufs=2)
        # s = 2G - x2_j (in place over gsb); then top-16 via max8/match_replace
        for mt in range(NI):
            nc.vector.scalar_tensor_tensor(
                out=gsb[mt], in0=gsb[mt], scalar=2.0, in1=x2b,
                op0=OP.mult, op1=OP.subtract)
            va = sb.tile([128, 8], F32, name="va", tag="va", bufs=2)
            vb = sb.tile([128, 8], F32, name="vb", tag="vb", bufs=2)
            idxu = sb.tile([128, K], U32, name="idxu", tag="idxu", bufs=2)
            idxf = None
            if not USE_DMA_GATHER:
                idxf = sb.tile([128, K], F16, name="idxf", tag="idxf", bufs=2)
            nc.vector.max(out=va, in_=gsb[mt])
            nc.vector.max_index(out=idxu[:, 0:8], in_max=va, in_values=gsb[mt])
            nc.vector.match_replace(out=gsb[mt], in_to_replace=va,
                                    in_values=gsb[mt], imm_value=NEG)
            nc.vector.max(out=vb, in_=gsb[mt])
            nc.vector.max_index(out=idxu[:, 8:16], in_max=vb, in_values=gsb[mt])
            if USE_DMA_GATHER:
                idx16 = sb.tile([128, K], F16, name="idx16", tag="idx16", bufs=2)
                nc.vector.tensor_copy(out=idx16, in_=idxu)
                pti = ptr.tile([K, 128], F16, name="pti", tag="ptr")
                nc.tensor.transpose(pti, idx16, identw16)
                nc.vector.tensor_copy(
                    out=idxg[0:K, mt * 128:(mt + 1) * 128], in_=pti)
            else:
                nc.vector.tensor_copy(out=idxf, in_=idxu)
                nc.sync.dma_start(out=irow_dram[mt * 128 * K:(mt + 1) * 128 * K], in_=idxf)

        if USE_DMA_GATHER:
            for g in range(1, 8):
                nc.sync.dma_start(out=idxg[g * K:(g + 1) * K, :], in_=idxg[0:K, :])
            return idxg
        irow_b = sb.tile([128, P * K], F16, name=f"irow_b{conv_id}", tag="irow_b")
        nc.sync.dma_start(out=irow_b, in_=irow_dram.rearrange("(o n) -> o n", o=1).to_broadcast([128, P * K]))
        return irow_b

    def onehot_chunk(irow_b, ch, conv_id):
        ohs = []
        for jt in range(NI):
            oh = sb.tile([128, CH], F16, name="oh", tag=f"oh{jt}", bufs=2)
            nc.vector.tensor_scalar(
                oh, irow_b[:, ch * CH:(ch + 1) * CH], jc[jt], None, op0=OP.is_equal)
            ohs.append(oh)
        return ohs

    # ==================================================================
    # conv1
    # ==================================================================
    irow_b1 = gram_topk_idxrow(xt32, 8, "gsb", t["irow1"], t["xrow1"], 1)

    # c1 = X @ (Wtop-Wbot) + b1  (feature-major), b1tok = X @ Wbot (token-major)
    c1T = []
    for mt in range(NI):
        pc = pmm.tile([128, P], F32, name="pc", tag="pmm")
        for kt in range(8):
            nc.tensor.matmul(pc, lhsT=wA1(kt, mt), rhs=xt16[kt],
                             start=(kt == 0), stop=(kt == 7))
        c = sb.tile([128, P], F32, name=f"c1T{mt}", tag=f"cT{mt}")
        nc.vector.tensor_scalar(c, pc, b1c[mt], None, op0=OP.add)
        c1T.append(c)
    btok1 = sb.tile([128, NI * 512], F16, name="btok1", tag="btok")
    for jt in range(NI):
        pb = pmm.tile([128, 512], F32, name="pb", tag="pmm")
        for kt in range(8):
            nc.tensor.matmul(pb, lhsT=xt16[kt][:, jt * 128:(jt + 1) * 128],
                             rhs=wA2(kt), start=(kt == 0), stop=(kt == 7))
        nc.vector.tensor_copy(out=btok1[:, jt * 512:(jt + 1) * 512], in_=pb)

    # gather chunks: T[c, i] = max_r b[knn_r(i), c]
    Tt = [sb.tile([128, P], F32, name=f"Tt{ct}", tag=f"xt32_{ct}") for ct in range(NI)]
    IC1 = CH // K
    for ch in range(NCH):
        if USE_DMA_GATHER:
            gout = sb.tile([128, NI, CH], F16, name="gout1", tag="gout", bufs=3)
            nc.gpsimd.dma_gather(
                out_ap=gout, in_ap=btok1,
                idxs_ap=irow_b1[:, ch * IC1:(ch + 1) * IC1],
                num_idxs=CH, num_idxs_reg=CH, elem_size=512,
                transpose=True, queue_num=0,
                sbuf_tokens_per_rank=128, sbuf_free_dim_per_rank=1024)
            for ct in range(NI):
                nc.vector.tensor_reduce(
                    out=Tt[ct][:, ch * IC1:(ch + 1) * IC1],
                    in_=gout[:, ct, :].rearrange("p (i r) -> p i r", r=K),
                    axis=mybir.AxisListType.X, op=OP.max)
        else:
            ohs = onehot_chunk(irow_b1, ch, 1)
            for ct in range(NI):
                pgt = pmm.tile([128, CH], F32, name="pgt", tag="pmm")
                for jt in range(NI):
                    nc.tensor.matmul(pgt, lhsT=btok1[:, jt * 512 + ct * 128:jt * 512 + (ct + 1) * 128],
                                     rhs=ohs[jt], start=(jt == 0), stop=(jt == NI - 1))
                nc.vector.tensor_reduce(
                    out=Tt[ct][:, ch * IC1:(ch + 1) * IC1],
                    in_=pgt.rearrange("p (i r) -> p i r", r=K),
                    axis=mybir.AxisListType.X, op=OP.max)

    # x1 = g1 * relu(c1 + T) + bt1   (keep fp32 for Gram, fp16 for matmuls)
    x1_32 = []
    x1_16 = []
    for ct in range(NI):
        nc.vector.tensor_add(c1T[ct], c1T[ct], Tt[ct])
        x32 = sb.tile([128, P], F32, name=f"x1_32_{ct}", tag=f"xt32_{4 + ct}")
        nc.scalar.activation(out=x32, in_=c1T[ct], func=AF.Relu)
        nc.vector.tensor_scalar(x32, x32, g1c[ct], bt1c[ct], op0=OP.mult, op1=OP.add)
        x16 = sb.tile([128, P], F16, name=f"x1_16_{ct}", tag=f"x1_16_{ct}")
        nc.vector.tensor_copy(out=x16, in_=x32)
        x1_32.append(x32)
        x1_16.append(x16)

    # ==================================================================
    # conv2
    # ==================================================================
    irow_b2 = gram_topk_idxrow(x1_32, NI, "gsb", t["irow2"], t["xrow2"], 2)

    c2T = []
    for mt in range(NI):
        pc2 = pmm.tile([128, P], F32, name="pc2", tag="pmm")
        for kt in range(NI):
            nc.tensor.matmul(pc2, lhsT=wB_c2(kt, mt), rhs=x1_16[kt],
                             start=(kt == 0), stop=(kt == NI - 1))
        c2 = sb.tile([128, P], F32, name=f"c2T{mt}", tag=f"cT{mt}")
        nc.vector.tensor_scalar(c2, pc2, b21c[mt], None, op0=OP.add)
        c2T.append(c2)
    btok2 = sb.tile([128, NI * 512], F16, name="btok2", tag="btok")
    for jt in range(NI):
        pb2 = pmm.tile([128, 512], F32, name="pb2", tag="pmm")
        for kt in range(NI):
            nc.tensor.matmul(pb2, lhsT=x1_16[kt][:, jt * 128:(jt + 1) * 128],
                             rhs=wB_b2(kt), start=(kt == 0), stop=(kt == NI - 1))
        nc.vector.tensor_copy(out=btok2[:, jt * 512:(jt + 1) * 512], in_=pb2)

    x2acc = [sb.tile([128, P], F32, name=f"x2acc{nt}", tag=f"xt32_{nt}") for nt in range(8)]
    IC = CH // K  # 32 points per chunk
    for ch in range(NCH):
        h1 = []
        if USE_DMA_GATHER:
            gout2 = sb.tile([128, NI, CH], F16, name="gout2", tag="gout", bufs=3)
            nc.gpsimd.dma_gather(
                out_ap=gout2, in_ap=btok2,
                idxs_ap=irow_b2[:, ch * IC:(ch + 1) * IC],
                num_idxs=CH, num_idxs_reg=CH, elem_size=512,
                transpose=True, queue_num=0,
                sbuf_tokens_per_rank=128, sbuf_free_dim_per_rank=1024)
            for ct in range(NI):
                h = sb.tile([128, CH], F16, name="h1", tag=f"h1_{ct}", bufs=2)
                cbc = c2T[ct][:, ch * IC:(ch + 1) * IC].rearrange(
                    "p (i o) -> p i o", o=1).to_broadcast([128, IC, K])
                nc.vector.tensor_tensor(
                    out=h.rearrange("p (i r) -> p i r", r=K),
                    in0=gout2[:, ct, :].rearrange("p (i r) -> p i r", r=K),
                    in1=cbc, op=OP.add)
                nc.scalar.activation(out=h, in_=h, func=AF.Relu)
                h1.append(h)
        else:
            ohs = onehot_chunk(irow_b2, ch, 2)
            for ct in range(NI):
                pg2 = pmm.tile([128, CH], F32, name="pg2", tag="pmm")
                for jt in range(NI):
                    nc.tensor.matmul(pg2, lhsT=btok2[:, jt * 512 + ct * 128:jt * 512 + (ct + 1) * 128],
                                     rhs=ohs[jt], start=(jt == 0), stop=(jt == NI - 1))
                h = sb.tile([128, CH], F16, name="h1", tag=f"h1_{ct}", bufs=2)
                cbc = c2T[ct][:, ch * IC:(ch + 1) * IC].rearrange(
                    "p (i o) -> p i o", o=1).to_broadcast([128, IC, K])
                nc.vector.tensor_tensor(
                    out=h.rearrange("p (i r) -> p i r", r=K),
                    in0=pg2.rearrange("p (i r) -> p i r", r=K),
                    in1=cbc, op=OP.add)
                nc.scalar.activation(out=h, in_=h, func=AF.Relu)
                h1.append(h)
        # layer2: h2 = relu(h1 @ W22f + b22f)
        h2 = []
        for nt in range(8):
            pz2 = pmm.tile([128, CH], F32, name="pz2", tag="pmm")
            for kt in range(NI):
                nc.tensor.matmul(pz2, lhsT=wB_22(kt, nt), rhs=h1[kt],
                                 start=(kt == 0), stop=(kt == NI - 1))
            hh = sb.tile([128, CH], F16, name="h2", tag=f"h2_{nt}", bufs=2)
            nc.scalar.activation(out=hh, in_=pz2, func=AF.Relu, bias=b22c[nt])
            h2.append(hh)
        # layer3 + max over ranks
        for nt in range(8):
            pz3 = pmm.tile([128, CH], F32, name="pz3", tag="pmm")
            for kt in range(8):
                nc.tensor.matmul(pz3, lhsT=wB_23(kt, nt), rhs=h2[kt],
                                 start=(kt == 0), stop=(kt == 7))
            nc.vector.tensor_reduce(
                out=x2acc[nt][:, ch * IC:(ch + 1) * IC],
                in_=pz3.rearrange("p (i r) -> p i r", r=K),
                axis=mybir.AxisListType.X, op=OP.max)

    # x2relu = relu(x2acc + b23f) -> fp16
    x2r = []
    for nt in range(8):
        xx = sb.tile([128, P], F16, name=f"x2r{nt}", tag=f"x2r{nt}")
        nc.scalar.activation(out=xx, in_=x2acc[nt], func=AF.Relu, bias=b23c[nt])
        x2r.append(xx)

    # ---- weight slab C: lin1 (reuses slab A space) ----
    slabC = wa.tile([128, 12 * 2048], F16, name="slabC", tag="wslabA")
    for kt in range(4):
        nc.sync.dma_start(out=slabC[:, kt * 2048:(kt + 1) * 2048],
                          in_=t["wlt"][kt * 128:(kt + 1) * 128, :])
    for kt in range(8):
        nc.sync.dma_start(out=slabC[:, (4 + kt) * 2048:(5 + kt) * 2048],
                          in_=t["wlb"][kt * 128:(kt + 1) * 128, :])

    # lin1 + global max pool -> pooled [2048] (fp16 cols)
    pooled = []
    for mt in range(16):
        pzl = pmm.tile([128, P], F32, name="pzl", tag="pmm")
        for kt in range(4):
            nc.tensor.matmul(
                pzl, lhsT=slabC[:, kt * 2048 + mt * 128: kt * 2048 + (mt + 1) * 128],
                rhs=x1_16[kt], start=(kt == 0), stop=False)
        for kt in range(8):
            nc.tensor.matmul(
                pzl, lhsT=slabC[:, (4 + kt) * 2048 + mt * 128: (4 + kt) * 2048 + (mt + 1) * 128],
                rhs=x2r[kt], start=False, stop=(kt == 7))
        pcol = sb.tile([128, 1], F32, name="pcol", tag="pcol", bufs=4)
        nc.vector.tensor_reduce(out=pcol, in_=pzl, axis=mybir.AxisListType.X, op=OP.max)
        pc16 = sb.tile([128, 1], F16, name=f"pool{mt}", tag=f"pool{mt}")
        nc.scalar.activation(out=pc16, in_=pcol, func=AF.Relu, bias=blc[mt])
        pooled.append(pc16)

    # ---- weight slab D: heads (reuses slab B space) ----
    slabD = wb.tile([128, 16 * 1024 + 8 * 512 + 4 * 16], F16, name="slabD", tag="wslabB")
    oD2 = 16 * 1024
    oD3 = oD2 + 8 * 512
    for kt in range(16):
        nc.sync.dma_start(out=slabD[:, kt * 1024:(kt + 1) * 1024],
                          in_=t["wh1"][kt * 128:(kt + 1) * 128, :])
    for kt in range(8):
        nc.sync.dma_start(out=slabD[:, oD2 + kt * 512:oD2 + (kt + 1) * 512],
                          in_=t["wh2"][kt * 128:(kt + 1) * 128, :])
    for kt in range(4):
        nc.sync.dma_start(out=slabD[:, oD3 + kt * 16:oD3 + kt * 16 + 10],
                          in_=t["whw"][kt * 128:(kt + 1) * 128, :])

    # head1
    hh1 = []
    for mt in range(8):
        ph1 = ptr.tile([128, 1], F32, name="ph1", tag="ptr")
        for kt in range(16):
            nc.tensor.matmul(
                ph1, lhsT=slabD[:, kt * 1024 + mt * 128: kt * 1024 + (mt + 1) * 128],
                rhs=pooled[kt], start=(kt == 0), stop=(kt == 15))
        h1c = sb.tile([128, 1], F16, name=f"hh1_{mt}", tag=f"hh1_{mt}")
        nc.scalar.activation(out=h1c, in_=ph1, func=AF.Relu, bias=bh1c[mt])
        hh1.append(h1c)
    # head2
    hh2 = []
    for mt in range(4):
        ph2 = ptr.tile([128, 1], F32, name="ph2", tag="ptr")
        for kt in range(8):
            nc.tensor.matmul(
                ph2, lhsT=slabD[:, oD2 + kt * 512 + mt * 128: oD2 + kt * 512 + (mt + 1) * 128],
                rhs=hh1[kt], start=(kt == 0), stop=(kt == 7))
        h2c = sb.tile([128, 1], F16, name=f"hh2_{mt}", tag=f"hh2_{mt}")
        nc.scalar.activation(out=h2c, in_=ph2, func=AF.Relu, bias=bh2c[mt])
        hh2.append(h2c)
    # final linear
    po = ptr.tile([10, 1], F32, name="po", tag="ptr")
    for kt in range(4):
        nc.tensor.matmul(po, lhsT=slabD[:, oD3 + kt * 16:oD3 + kt * 16 + 10],
                         rhs=hh2[kt], start=(kt == 0), stop=(kt == 3))
    osb = sb.tile([10, 1], F32, name="osb", tag="osb")
    nc.vector.tensor_scalar(osb, po, bwc[0][:10], None, op0=OP.add)
    nc.sync.dma_start(out=t["out"], in_=osb)


def _build_program():
    if "prog" in _PROG_CACHE:
        return _PROG_CACHE["prog"]
    from contextlib import ExitStack
    nc = bacc.Bacc("TRN2", target_bir_lowering=False, debug=False,
                   num_devices=N_CORES)
    t = _declare(nc)
    with tile.TileContext(nc) as tc:
        with ExitStack() as ctx:
            _emit(ctx, tc, t)
    nc.compile()
    _PROG_CACHE["prog"] = nc
    return nc


# --------------------------------------------------------------------------
# host side
# --------------------------------------------------------------------------

def _fold_params(params):
    def f32(a):
        return np.asarray(a, np.float32)

    def f64(a):
        return np.asarray(a, np.float64)

    (W1, b1, g1, bt1), = [tuple(map(f32, l)) for l in params["conv1"]]
    conv2 = [tuple(map(f32, l)) for l in params["conv2"]]
    (W21, b21, g21, bt21), (W22, b22, g22, bt22), (W23, b23, g23, bt23) = conv2
    (Wl, bl, gl, btl), = [tuple(map(f32, l)) for l in params["lin1"]]
    (Wh1, bh1, gh1, bth1), = [tuple(map(f32, l)) for l in params["head1"]]
    (Wh2, bh2, gh2, bth2), = [tuple(map(f32, l)) for l in params["head2"]]
    WhW = f32(params["headW"])
    bhW = f32(params["headb"])

    ok = min(g1.min(), g23.min(), gl.min()) > 0
    d = {
        "wc1": W1[:1024] - W1[1024:], "wb1": W1[1024:],
        "b1v": b1, "g1v": g1, "bt1v": bt1,
        "wc2": W21[:512] - W21[512:], "wb2": W21[512:], "b21v": b21,
        "w22": g21[:, None] * W22,
        "b22v": (f64(bt21) @ f64(W22) + b22).astype(np.float32),
        "w23": g22[:, None] * W23,
        "b23v": (f64(bt22) @ f64(W23) + b23).astype(np.float32),
        "wlt": Wl[:512], "wlb": g23[:, None] * Wl[512:],
        "blv": (f64(bl) + f64(bt23) @ f64(Wl[512:])).astype(np.float32),
        "wh1": gl[:, None] * Wh1,
        "bh1v": (f64(btl) @ f64(Wh1) + bh1).astype(np.float32),
        "wh2": gh1[:, None] * Wh2,
        "bh2v": (f64(bth1) @ f64(Wh2) + bh2).astype(np.float32),
        "whw": gh2[:, None] * WhW,
        "bwv": (f64(bth2) @ f64(WhW) + bhW).astype(np.float32),
    }
    casts = {"wc1", "wb1", "wc2", "wb2", "w22", "w23", "wlt", "wlb",
             "wh1", "wh2", "whw"}
    out = {}
    for k, v in d.items():
        out[k] = np.ascontiguousarray(
            v.astype(np.float16) if k in casts else v.astype(np.float32))
    out["jidx"] = np.arange(512, dtype=np.float32)
    out["ident"] = np.eye(128, dtype=np.float32)
    out["ident16"] = np.eye(128, dtype=np.float16)
    return out, ok


def _reference_numpy(x, params):
    """Exact eval-mode fallback (never used for the shipped seed, where all
    fold-relevant BN scales are positive)."""
    x = np.asarray(x, np.float32)
    B = 8
    Pn = x.shape[0] // B
    xg = x.reshape(B, Pn, -1)

    def mlp(a, layers):
        for (W, b, g, bt) in layers:
            a = np.maximum(a @ np.asarray(W, np.float32) + np.asarray(b, np.float32), 0)
            a = a * np.asarray(g, np.float32) + np.asarray(bt, np.float32)
        return a

    def edge_conv(a, layers):
        x2 = (a * a).sum(-1)
        d2 = x2[:, :, None] + x2[:, None, :] - 2.0 * np.einsum("bic,bjc->bij", a, a)
        idx = np.argsort(d2, axis=2, kind="stable")[:, :, :K]
        out = []
        for bi in range(B):
            xj = a[bi][idx[bi]]
            xi = np.broadcast_to(a[bi][:, None, :], xj.shape)
            feat = np.concatenate([xi, xj - xi], -1)
            out.append(mlp(feat, layers).max(1))
        return np.stack(out)

    x1 = edge_conv(xg, params["conv1"])
    x2_ = edge_conv(x1, params["conv2"])
    o = mlp(np.concatenate([x1, x2_], -1), params["lin1"]).max(1)
    o = mlp(o, params["head1"])
    o = mlp(o, params["head2"])
    return (o @ np.asarray(params["headW"], np.float32)
            + np.asarray(params["headb"], np.float32)).astype(np.float32)


def kernel(x, batch=None, num_graphs=None, params=None, **_):
    x = np.ascontiguousarray(np.asarray(x, np.float32))
    folded, ok = _fold_params(params)
    if not ok:
        return _reference_numpy(x, params)

    nc = _build_program()
    in_maps = []
    for c in range(N_CORES):
        m = dict(folded)
        m["xg"] = x[c * P:(c + 1) * P]
        in_maps.append(m)
    res = run_bass_kernel_spmd(nc, in_maps, list(range(N_CORES)))
    return np.stack([res.results[c]["out"] for c in range(N_CORES)]).astype(np.float32)


if __name__ == "__main__":
    _build_program()
    print("program built + compiled OK")


# revision 13
# speedup vs baseline: 15.6418x; 1.0060x over previous
"""Trainium2 Bass kernel for nn_DynamicModel_79955111182517 (DGCNN-style
EdgeConv GNN, B=8 graphs x P=512 points x C=1024, k=16 kNN).

Strategy (data-parallel, one graph per NeuronCore):
- BatchNorm (eval) scale/shift folded into adjacent matmuls on the host;
  max-aggregations commute with the positive per-channel scales.
- EdgeConv message nn([x_i, x_j - x_i]) is decomposed: feat @ W =
  x_i @ (Wtop - Wbot) + x_j @ Wbot, so only two P x C matmuls are needed
  instead of P*k x 2C.
- conv1 (single-layer MLP) factorizes fully: max_k relu(c_i + b_j) =
  relu(c_i + max_{j in knn(i)} b_j).
- kNN: fp32 Gram matrix on the PE, top-16 per row via the DVE max8 /
  max_index / match_replace ISA ops; neighbor gathers are one-hot fp16
  matmuls on the PE (keeps everything feature-major for the MLP chain).
- All heavy matmuls run in fp16 (full PE rate, ~8x finer rounding than
  bf16); distances in fp32 so neighbor selection matches the reference.
"""

import sys
import numpy as np

try:
    import concourse.bass as bass  # noqa: F401
except ImportError:
    sys.path.insert(0, "/opt/trn_rl_repo")

import concourse.bass as bass
import concourse.tile as tile
from concourse import bacc, mybir
from concourse.bass_utils import run_bass_kernel_spmd

F32 = mybir.dt.float32
F16 = mybir.dt.float16
U32 = mybir.dt.uint32
I16 = mybir.dt.int16
AF = mybir.ActivationFunctionType
OP = mybir.AluOpType

N_CORES = 8
P = 512          # points per graph
C0 = 1024        # input channels
K = 16           # neighbors
NI = P // 128    # 4 token tiles
CH = 512         # gather-chunk columns (32 points x 16 ranks)
NCH = (P * K) // CH  # 16 chunks
NEG = -1.0e30
USE_DMA_GATHER = True

_PROG_CACHE = {}


# --------------------------------------------------------------------------
# device program
# --------------------------------------------------------------------------

def _declare(nc):
    t = {}

    def inp(name, shape, dt):
        t[name] = nc.dram_tensor(name, list(shape), dt, kind="ExternalInput").ap()

    inp("xg", [P, C0], F32)
    inp("wc1", [1024, 512], F16)
    inp("wb1", [1024, 512], F16)
    inp("b1v", [512], F32)
    inp("g1v", [512], F32)
    inp("bt1v", [512], F32)
    inp("wc2", [512, 512], F16)
    inp("wb2", [512, 512], F16)
    inp("b21v", [512], F32)
    inp("w22", [512, 1024], F16)
    inp("b22v", [1024], F32)
    inp("w23", [1024, 1024], F16)
    inp("b23v", [1024], F32)
    inp("wlt", [512, 2048], F16)
    inp("wlb", [1024, 2048], F16)
    inp("blv", [2048], F32)
    inp("wh1", [2048, 1024], F16)
    inp("bh1v", [1024], F32)
    inp("wh2", [1024, 512], F16)
    inp("bh2v", [512], F32)
    inp("whw", [512, 10], F16)
    inp("bwv", [10], F32)
    inp("jidx", [512], F32)         # arange(512) for one-hot compares
    inp("ident", [128, 128], F32)   # PE-transpose identity
    inp("ident16", [128, 128], F16)

    t["out"] = nc.dram_tensor("out", [10], F32, kind="ExternalOutput").ap()
    # internal DRAM bounce buffers (partition -> row reshuffles)
    t["irow1"] = nc.dram_tensor("irow1", [P * K], F16).ap()
    t["irow2"] = nc.dram_tensor("irow2", [P * K], F16).ap()
    t["xrow1"] = nc.dram_tensor("xrow1", [P], F32).ap()
    t["xrow2"] = nc.dram_tensor("xrow2", [P], F32).ap()
    return t


def _make_pools(ctx, tc):
    return dict(
        wa=ctx.enter_context(tc.tile_pool(name="wa", bufs=1)),
        wb=ctx.enter_context(tc.tile_pool(name="wb", bufs=1)),
        sb=ctx.enter_context(tc.tile_pool(name="sb", bufs=1)),
        cols=ctx.enter_context(tc.tile_pool(name="cols", bufs=1)),
        pmm=ctx.enter_context(tc.tile_pool(name="pmm", bufs=6, space="PSUM")),
        ptr=ctx.enter_context(tc.tile_pool(name="ptr", bufs=2, space="PSUM")),
    )


def _emit(ctx, tc, t, pools=None):
    nc = tc.nc
    if pools is None:
        pools = _make_pools(ctx, tc)
    wa, wb, sb, cols, pmm, ptr = (pools[k] for k in
                                  ("wa", "wb", "sb", "cols", "pmm", "ptr"))

    def col_load(vec_ap, n128, base_name, dt=F32):
        out = []
        for i in range(n128):
            n = min(128, vec_ap.shape[0] - i * 128)
            c = cols.tile([n, 1], dt, name=f"{base_name}{i}", tag=f"{base_name}{i}")
            nc.sync.dma_start(out=c, in_=vec_ap[i * 128:i * 128 + n])
            out.append(c)
        return out

    # ---- constant columns ----
    b1c = col_load(t["b1v"], 4, "b1c")
    g1c = col_load(t["g1v"], 4, "g1c")
    bt1c = col_load(t["bt1v"], 4, "bt1c")
    b21c = col_load(t["b21v"], 4, "b21c")
    b22c = col_load(t["b22v"], 8, "b22c")
    b23c = col_load(t["b23v"], 8, "b23c")
    blc = col_load(t["blv"], 16, "blc")
    bh1c = col_load(t["bh1v"], 8, "bh1c")
    bh2c = col_load(t["bh2v"], 4, "bh2c")
    bwc = col_load(t["bwv"], 1, "bwc")
    jc = col_load(t["jidx"], 4, "jc", dt=F32)
    identw = cols.tile([128, 128], F32, name="identw", tag="identw")
    nc.sync.dma_start(out=identw, in_=t["ident"])
    identw16 = cols.tile([128, 128], F16, name="identw16", tag="identw16")
    nc.sync.dma_start(out=identw16, in_=t["ident16"])

    # ==================================================================
    # stage 0/1: load X blockwise, transpose on PE -> XT (fp32 + fp16)
    # ==================================================================
    xt32 = [sb.tile([128, P], F32, name=f"xt32_{ct}", tag=f"xt32_{ct}") for ct in range(8)]
    xt16 = [sb.tile([128, P], F16, name=f"xt16_{ct}", tag=f"xt16_{ct}") for ct in range(8)]
    for ct in range(8):
        for it in range(NI):
            xb = sb.tile([128, 128], F32, name="xb", tag="xblk", bufs=6)
            nc.sync.dma_start(
                out=xb, in_=t["xg"][it * 128:(it + 1) * 128, ct * 128:(ct + 1) * 128])
            pt = ptr.tile([128, 128], F32, name="pt", tag="ptr")
            nc.tensor.transpose(pt, xb, identw)
            nc.scalar.activation(out=xt32[ct][:, it * 128:(it + 1) * 128], in_=pt, func=AF.Copy)
        nc.vector.tensor_copy(out=xt16[ct], in_=xt32[ct])

    # ---- weight slab A: conv1 (later reused for lin1) ----
    # layout: [wc1 8kt x 512 | wb1 8kt x 512]
    slabA = wa.tile([128, 16 * 512], F16, name="slabA", tag="wslabA")
    for kt in range(8):
        nc.scalar.dma_start(out=slabA[:, kt * 512:(kt + 1) * 512],
                          in_=t["wc1"][kt * 128:(kt + 1) * 128, :])
        nc.scalar.dma_start(out=slabA[:, (8 + kt) * 512:(9 + kt) * 512],
                          in_=t["wb1"][kt * 128:(kt + 1) * 128, :])

    def wA1(kt, mt):  # wc1 lhsT block [128k, 128m]
        return slabA[:, kt * 512 + mt * 128: kt * 512 + (mt + 1) * 128]

    def wA2(kt):      # wb1 rhs k-tile [128k, 512]
        return slabA[:, (8 + kt) * 512:(9 + kt) * 512]

    # ---- weight slab B: conv2 (later reused for heads) ----
    # layout: [wc2 4x512 | wb2 4x512 | w22 4x1024 | w23 8x1024]
    slabB = wb.tile([128, 2048 + 2048 + 4096 + 8192], F16, name="slabB", tag="wslabB")
    oB2 = 2048
    oB3 = 4096
    oB4 = 8192
    for kt in range(4):
        nc.scalar.dma_start(out=slabB[:, kt * 512:(kt + 1) * 512],
                          in_=t["wc2"][kt * 128:(kt + 1) * 128, :])
        nc.scalar.dma_start(out=slabB[:, oB2 + kt * 512:oB2 + (kt + 1) * 512],
                          in_=t["wb2"][kt * 128:(kt + 1) * 128, :])
        nc.scalar.dma_start(out=slabB[:, oB3 + kt * 1024:oB3 + (kt + 1) * 1024],
                          in_=t["w22"][kt * 128:(kt + 1) * 128, :])
    for kt in range(8):
        nc.scalar.dma_start(out=slabB[:, oB4 + kt * 1024:oB4 + (kt + 1) * 1024],
                          in_=t["w23"][kt * 128:(kt + 1) * 128, :])

    def wB_c2(kt, mt):
        return slabB[:, kt * 512 + mt * 128: kt * 512 + (mt + 1) * 128]

    def wB_b2(kt):
        return slabB[:, oB2 + kt * 512:oB2 + (kt + 1) * 512]

    def wB_22(kt, nt):
        return slabB[:, oB3 + kt * 1024 + nt * 128: oB3 + kt * 1024 + (nt + 1) * 128]

    def wB_23(kt, nt):
        return slabB[:, oB4 + kt * 1024 + nt * 128: oB4 + kt * 1024 + (nt + 1) * 128]

    # ==================================================================
    # generic helpers
    # ==================================================================
    def gram_topk_idxrow(xt_f32, nk, gsb_tag, irow_dram, xrow_dram, conv_id):
        """Gram matrix (fp32), scores s = 2G - x2_j, top-16 indices per row,
        assembled into a broadcast index row [128, P*K] (f16 values)."""
        gsb = []
        for mt in range(NI):
            pg = pmm.tile([128, P], F32, name="pg", tag="pmm")
            for kt in range(nk):
                nc.tensor.matmul(pg, lhsT=xt_f32[kt][:, mt * 128:(mt + 1) * 128],
                                 rhs=xt_f32[kt], start=(kt == 0), stop=(kt == nk - 1))
            g = sb.tile([128, P], F32, name=f"g{conv_id}_{mt}", tag=f"{gsb_tag}{mt}")
            nc.scalar.activation(out=g, in_=pg, func=AF.Copy)
            gsb.append(g)

        # x2 row from the diagonal of G
        for mt in range(NI):
            dsel = sb.tile([128, P], F32, name="dsel", tag="dsel", bufs=2)
            nc.gpsimd.affine_select(
                out=dsel, in_=gsb[mt], compare_op=OP.is_equal, fill=0.0,
                base=128 * mt, pattern=[[-1, P]], channel_multiplier=1)
            dcol = sb.tile([128, 1], F32, name="dcol", tag="dcol", bufs=2)
            nc.vector.reduce_sum(out=dcol, in_=dsel, axis=mybir.AxisListType.X)
            nc.sync.dma_start(out=xrow_dram[mt * 128:(mt + 1) * 128], in_=dcol)
        x2b = sb.tile([128, P], F32, name=f"x2b{conv_id}", tag="x2b")
        nc.sync.dma_start(out=x2b, in_=xrow_dram.rearrange("(o n) -> o n", o=1).to_broadcast([128, P]))

        idxg = None
        if USE_DMA_GATHER:
            idxg = sb.tile([128, P], I16, name=f"idxg{conv_id}", tag="idxg", bufs=2)
        # s = 2G - x2_j (in place over gsb); then top-16 via max8/match_replace
        for mt in range(NI):
            nc.vector.scalar_tensor_tensor(
                out=gsb[mt], in0=gsb[mt], scalar=2.0, in1=x2b,
                op0=OP.mult, op1=OP.subtract)
            va = sb.tile([128, 8], F32, name="va", tag="va", bufs=2)
            vb = sb.tile([128, 8], F32, name="vb", tag="vb", bufs=2)
            idxu = sb.tile([128, K], U32, name="idxu", tag="idxu", bufs=2)
            idxf = None
            if not USE_DMA_GATHER:
                idxf = sb.tile([128, K], F16, name="idxf", tag="idxf", bufs=2)
            nc.vector.max(out=va, in_=gsb[mt])
            nc.vector.max_index(out=idxu[:, 0:8], in_max=va, in_values=gsb[mt])
            nc.vector.match_replace(out=gsb[mt], in_to_replace=va,
                                    in_values=gsb[mt], imm_value=NEG)
            nc.vector.max(out=vb, in_=gsb[mt])
            nc.vector.max_index(out=idxu[:, 8:16], in_max=vb, in_values=gsb[mt])
            if USE_DMA_GATHER:
                idx16 = sb.tile([128, K], F16, name="idx16", tag="idx16", bufs=2)
                nc.vector.tensor_copy(out=idx16, in_=idxu)
                pti = ptr.tile([K, 128], F16, name="pti", tag="ptr")
                nc.tensor.transpose(pti, idx16, identw16)
                nc.vector.tensor_copy(
                    out=idxg[0:K, mt * 128:(mt + 1) * 128], in_=pti)
            else:
                nc.vector.tensor_copy(out=idxf, in_=idxu)
                nc.sync.dma_start(out=irow_dram[mt * 128 * K:(mt + 1) * 128 * K], in_=idxf)

        if USE_DMA_GATHER:
            for g in range(1, 8):
                nc.sync.dma_start(out=idxg[g * K:(g + 1) * K, :], in_=idxg[0:K, :])
            return idxg
        irow_b = sb.tile([128, P * K], F16, name=f"irow_b{conv_id}", tag="irow_b")
        nc.sync.dma_start(out=irow_b, in_=irow_dram.rearrange("(o n) -> o n", o=1).to_broadcast([128, P * K]))
        return irow_b

    def onehot_chunk(irow_b, ch, conv_id):
        ohs = []
        for jt in range(NI):
            oh = sb.tile([128, CH], F16, name="oh", tag=f"oh{jt}", bufs=2)
            nc.vector.tensor_scalar(
                oh, irow_b[:, ch * CH:(ch + 1) * CH], jc[jt], None, op0=OP.is_equal)
            ohs.append(oh)
        return ohs

    # ==================================================================
    # conv1
    # ==================================================================
    irow_b1 = gram_topk_idxrow(xt32, 8, "gsb", t["irow1"], t["xrow1"], 1)

    btok1 = sb.tile([128, NI * 512], F16, name="btok1", tag="btok")
    for jt in range(NI):
        pb = pmm.tile([128, 512], F32, name="pb", tag="pmm")
        for kt in range(8):
            nc.tensor.matmul(pb, lhsT=xt16[kt][:, jt * 128:(jt + 1) * 128],
                             rhs=wA2(kt), start=(kt == 0), stop=(kt == 7))
        nc.vector.tensor_copy(out=btok1[:, jt * 512:(jt + 1) * 512], in_=pb)

    # c1 = X @ (Wtop-Wbot) + b1  (feature-major), b1tok = X @ Wbot (token-major)
    c1T = []
    for mt in range(NI):
        pc = pmm.tile([128, P], F32, name="pc", tag="pmm")
        for kt in range(8):
            nc.tensor.matmul(pc, lhsT=wA1(kt, mt), rhs=xt16[kt],
                             start=(kt == 0), stop=(kt == 7))
        c = sb.tile([128, P], F32, name=f"c1T{mt}", tag=f"cT{mt}")
        nc.vector.tensor_scalar(c, pc, b1c[mt], None, op0=OP.add)
        c1T.append(c)
    # gather chunks: T[c, i] = max_r b[knn_r(i), c]
    Tt = [sb.tile([128, P], F32, name=f"Tt{ct}", tag=f"xt32_{ct}") for ct in range(NI)]
    IC1 = CH // K
    for ch in range(NCH):
        if USE_DMA_GATHER:
            gout = sb.tile([128, NI, CH], F16, name="gout1", tag="gout", bufs=3)
            nc.gpsimd.dma_gather(
                out_ap=gout, in_ap=btok1,
                idxs_ap=irow_b1[:, ch * IC1:(ch + 1) * IC1],
                num_idxs=CH, num_idxs_reg=CH, elem_size=512,
                transpose=True, queue_num=0,
                sbuf_tokens_per_rank=128, sbuf_free_dim_per_rank=1024)
            for ct in range(NI):
                nc.vector.tensor_reduce(
                    out=Tt[ct][:, ch * IC1:(ch + 1) * IC1],
                    in_=gout[:, ct, :].rearrange("p (i r) -> p i r", r=K),
                    axis=mybir.AxisListType.X, op=OP.max)
        else:
            ohs = onehot_chunk(irow_b1, ch, 1)
            for ct in range(NI):
                pgt = pmm.tile([128, CH], F32, name="pgt", tag="pmm")
                for jt in range(NI):
                    nc.tensor.matmul(pgt, lhsT=btok1[:, jt * 512 + ct * 128:jt * 512 + (ct + 1) * 128],
                                     rhs=ohs[jt], start=(jt == 0), stop=(jt == NI - 1))
                nc.vector.tensor_reduce(
                    out=Tt[ct][:, ch * IC1:(ch + 1) * IC1],
                    in_=pgt.rearrange("p (i r) -> p i r", r=K),
                    axis=mybir.AxisListType.X, op=OP.max)

    # x1 = g1 * relu(c1 + T) + bt1   (keep fp32 for Gram, fp16 for matmuls)
    x1_32 = []
    x1_16 = []
    for ct in range(NI):
        nc.vector.tensor_add(c1T[ct], c1T[ct], Tt[ct])
        x32 = sb.tile([128, P], F32, name=f"x1_32_{ct}", tag=f"xt32_{4 + ct}")
        nc.scalar.activation(out=x32, in_=c1T[ct], func=AF.Relu)
        nc.vector.tensor_scalar(x32, x32, g1c[ct], bt1c[ct], op0=OP.mult, op1=OP.add)
        x16 = sb.tile([128, P], F16, name=f"x1_16_{ct}", tag=f"x1_16_{ct}")
        nc.vector.tensor_copy(out=x16, in_=x32)
        x1_32.append(x32)
        x1_16.append(x16)

    # ==================================================================
    # conv2
    # ==================================================================
    irow_b2 = gram_topk_idxrow(x1_32, NI, "gsb", t["irow2"], t["xrow2"], 2)

    btok2 = sb.tile([128, NI * 512], F16, name="btok2", tag="btok")
    for jt in range(NI):
        pb2 = pmm.tile([128, 512], F32, name="pb2", tag="pmm")
        for kt in range(NI):
            nc.tensor.matmul(pb2, lhsT=x1_16[kt][:, jt * 128:(jt + 1) * 128],
                             rhs=wB_b2(kt), start=(kt == 0), stop=(kt == NI - 1))
        nc.vector.tensor_copy(out=btok2[:, jt * 512:(jt + 1) * 512], in_=pb2)

    c2T = []
    for mt in range(NI):
        pc2 = pmm.tile([128, P], F32, name="pc2", tag="pmm")
        for kt in range(NI):
            nc.tensor.matmul(pc2, lhsT=wB_c2(kt, mt), rhs=x1_16[kt],
                             start=(kt == 0), stop=(kt == NI - 1))
        c2 = sb.tile([128, P], F32, name=f"c2T{mt}", tag=f"cT{mt}")
        nc.vector.tensor_scalar(c2, pc2, b21c[mt], None, op0=OP.add)
        c2T.append(c2)
    x2acc = [sb.tile([128, P], F32, name=f"x2acc{nt}", tag=f"xt32_{nt}") for nt in range(8)]
    IC = CH // K  # 32 points per chunk
    for ch in range(NCH):
        h1 = []
        if USE_DMA_GATHER:
            gout2 = sb.tile([128, NI, CH], F16, name="gout2", tag="gout", bufs=3)
            nc.gpsimd.dma_gather(
                out_ap=gout2, in_ap=btok2,
                idxs_ap=irow_b2[:, ch * IC:(ch + 1) * IC],
                num_idxs=CH, num_idxs_reg=CH, elem_size=512,
                transpose=True, queue_num=0,
                sbuf_tokens_per_rank=128, sbuf_free_dim_per_rank=1024)
            for ct in range(NI):
                h = sb.tile([128, CH], F16, name="h1", tag=f"h1_{ct}", bufs=2)
                cbc = c2T[ct][:, ch * IC:(ch + 1) * IC].rearrange(
                    "p (i o) -> p i o", o=1).to_broadcast([128, IC, K])
                nc.vector.tensor_tensor(
                    out=h.rearrange("p (i r) -> p i r", r=K),
                    in0=gout2[:, ct, :].rearrange("p (i r) -> p i r", r=K),
                    in1=cbc, op=OP.add)
                nc.scalar.activation(out=h, in_=h, func=AF.Relu)
                h1.append(h)
        else:
            ohs = onehot_chunk(irow_b2, ch, 2)
            for ct in range(NI):
                pg2 = pmm.tile([128, CH], F32, name="pg2", tag="pmm")
                for jt in range(NI):
                    nc.tensor.matmul(pg2, lhsT=btok2[:, jt * 512 + ct * 128:jt * 512 + (ct + 1) * 128],
                                     rhs=ohs[jt], start=(jt == 0), stop=(jt == NI - 1))
                h = sb.tile([128, CH], F16, name="h1", tag=f"h1_{ct}", bufs=2)
                cbc = c2T[ct][:, ch * IC:(ch + 1) * IC].rearrange(
                    "p (i o) -> p i o", o=1).to_broadcast([128, IC, K])
                nc.vector.tensor_tensor(
                    out=h.rearrange("p (i r) -> p i r", r=K),
                    in0=pg2.rearrange("p (i r) -> p i r", r=K),
                    in1=cbc, op=OP.add)
                nc.scalar.activation(out=h, in_=h, func=AF.Relu)
                h1.append(h)
        # layer2: h2 = relu(h1 @ W22f + b22f)
        h2 = []
        for nt in range(8):
            pz2 = pmm.tile([128, CH], F32, name="pz2", tag="pmm")
            for kt in range(NI):
                nc.tensor.matmul(pz2, lhsT=wB_22(kt, nt), rhs=h1[kt],
                                 start=(kt == 0), stop=(kt == NI - 1))
            hh = sb.tile([128, CH], F16, name="h2", tag=f"h2_{nt}", bufs=2)
            nc.scalar.activation(out=hh, in_=pz2, func=AF.Relu, bias=b22c[nt])
            h2.append(hh)
        # layer3 + max over ranks
        for nt in range(8):
            pz3 = pmm.tile([128, CH], F32, name="pz3", tag="pmm")
            for kt in range(8):
                nc.tensor.matmul(pz3, lhsT=wB_23(kt, nt), rhs=h2[kt],
                                 start=(kt == 0), stop=(kt == 7))
            nc.vector.tensor_reduce(
                out=x2acc[nt][:, ch * IC:(ch + 1) * IC],
                in_=pz3.rearrange("p (i r) -> p i r", r=K),
                axis=mybir.AxisListType.X, op=OP.max)

    # x2relu = relu(x2acc + b23f) -> fp16
    x2r = []
    for nt in range(8):
        xx = sb.tile([128, P], F16, name=f"x2r{nt}", tag=f"x2r{nt}")
        nc.scalar.activation(out=xx, in_=x2acc[nt], func=AF.Relu, bias=b23c[nt])
        x2r.append(xx)

    # ---- weight slab C: lin1 (reuses slab A space) ----
    slabC = wa.tile([128, 12 * 2048], F16, name="slabC", tag="wslabA")
    for kt in range(4):
        nc.scalar.dma_start(out=slabC[:, kt * 2048:(kt + 1) * 2048],
                          in_=t["wlt"][kt * 128:(kt + 1) * 128, :])
    for kt in range(8):
        nc.scalar.dma_start(out=slabC[:, (4 + kt) * 2048:(5 + kt) * 2048],
                          in_=t["wlb"][kt * 128:(kt + 1) * 128, :])

    # lin1 + global max pool -> pooled [2048] (fp16 cols)
    pooled = []
    for mt in range(16):
        pzl = pmm.tile([128, P], F32, name="pzl", tag="pmm")
        for kt in range(4):
            nc.tensor.matmul(
                pzl, lhsT=slabC[:, kt * 2048 + mt * 128: kt * 2048 + (mt + 1) * 128],
                rhs=x1_16[kt], start=(kt == 0), stop=False)
        for kt in range(8):
            nc.tensor.matmul(
                pzl, lhsT=slabC[:, (4 + kt) * 2048 + mt * 128: (4 + kt) * 2048 + (mt + 1) * 128],
                rhs=x2r[kt], start=False, stop=(kt == 7))
        pcol = sb.tile([128, 1], F32, name="pcol", tag="pcol", bufs=4)
        nc.vector.tensor_reduce(out=pcol, in_=pzl, axis=mybir.AxisListType.X, op=OP.max)
        pc16 = sb.tile([128, 1], F16, name=f"pool{mt}", tag=f"pool{mt}")
        nc.scalar.activation(out=pc16, in_=pcol, func=AF.Relu, bias=blc[mt])
        pooled.append(pc16)

    # ---- weight slab D: heads (reuses slab B space) ----
    slabD = wb.tile([128, 16 * 1024 + 8 * 512 + 4 * 16], F16, name="slabD", tag="wslabB")
    oD2 = 16 * 1024
    oD3 = oD2 + 8 * 512
    for kt in range(16):
        nc.scalar.dma_start(out=slabD[:, kt * 1024:(kt + 1) * 1024],
                          in_=t["wh1"][kt * 128:(kt + 1) * 128, :])
    for kt in range(8):
        nc.scalar.dma_start(out=slabD[:, oD2 + kt * 512:oD2 + (kt + 1) * 512],
                          in_=t["wh2"][kt * 128:(kt + 1) * 128, :])
    for kt in range(4):
        nc.scalar.dma_start(out=slabD[:, oD3 + kt * 16:oD3 + kt * 16 + 10],
                          in_=t["whw"][kt * 128:(kt + 1) * 128, :])

    # head1
    hh1 = []
    for mt in range(8):
        ph1 = ptr.tile([128, 1], F32, name="ph1", tag="ptr")
        for kt in range(16):
            nc.tensor.matmul(
                ph1, lhsT=slabD[:, kt * 1024 + mt * 128: kt * 1024 + (mt + 1) * 128],
                rhs=pooled[kt], start=(kt == 0), stop=(kt == 15))
        h1c = sb.tile([128, 1], F16, name=f"hh1_{mt}", tag=f"hh1_{mt}")
        nc.scalar.activation(out=h1c, in_=ph1, func=AF.Relu, bias=bh1c[mt])
        hh1.append(h1c)
    # head2
    hh2 = []
    for mt in range(4):
        ph2 = ptr.tile([128, 1], F32, name="ph2", tag="ptr")
        for kt in range(8):
            nc.tensor.matmul(
                ph2, lhsT=slabD[:, oD2 + kt * 512 + mt * 128: oD2 + kt * 512 + (mt + 1) * 128],
                rhs=hh1[kt], start=(kt == 0), stop=(kt == 7))
        h2c = sb.tile([128, 1], F16, name=f"hh2_{mt}", tag=f"hh2_{mt}")
        nc.scalar.activation(out=h2c, in_=ph2, func=AF.Relu, bias=bh2c[mt])
        hh2.append(h2c)
    # final linear
    po = ptr.tile([10, 1], F32, name="po", tag="ptr")
    for kt in range(4):
        nc.tensor.matmul(po, lhsT=slabD[:, oD3 + kt * 16:oD3 + kt * 16 + 10],
                         rhs=hh2[kt], start=(kt == 0), stop=(kt == 3))
    osb = sb.tile([10, 1], F32, name="osb", tag="osb")
    nc.vector.tensor_scalar(osb, po, bwc[0][:10], None, op0=OP.add)
    nc.sync.dma_start(out=t["out"], in_=osb)


def _build_program():
    if "prog" in _PROG_CACHE:
        return _PROG_CACHE["prog"]
    from contextlib import ExitStack
    nc = bacc.Bacc("TRN2", target_bir_lowering=False, debug=False,
                   num_devices=N_CORES)
    t = _declare(nc)
    with tile.TileContext(nc) as tc:
        with ExitStack() as ctx:
            _emit(ctx, tc, t)
    nc.compile()
    _PROG_CACHE["prog"] = nc
    return nc


# --------------------------------------------------------------------------
# host side
# --------------------------------------------------------------------------

def _fold_params(params):
    def f32(a):
        return np.asarray(a, np.float32)

    def f64(a):
        return np.asarray(a, np.float64)

    (W1, b1, g1, bt1), = [tuple(map(f32, l)) for l in params["conv1"]]
    conv2 = [tuple(map(f32, l)) for l in params["conv2"]]
    (W21, b21, g21, bt21), (W22, b22, g22, bt22), (W23, b23, g23, bt23) = conv2
    (Wl, bl, gl, btl), = [tuple(map(f32, l)) for l in params["lin1"]]
    (Wh1, bh1, gh1, bth1), = [tuple(map(f32, l)) for l in params["head1"]]
    (Wh2, bh2, gh2, bth2), = [tuple(map(f32, l)) for l in params["head2"]]
    WhW = f32(params["headW"])
    bhW = f32(params["headb"])

    ok = min(g1.min(), g23.min(), gl.min()) > 0
    d = {
        "wc1": W1[:1024] - W1[1024:], "wb1": W1[1024:],
        "b1v": b1, "g1v": g1, "bt1v": bt1,
        "wc2": W21[:512] - W21[512:], "wb2": W21[512:], "b21v": b21,
        "w22": g21[:, None] * W22,
        "b22v": (f64(bt21) @ f64(W22) + b22).astype(np.float32),
        "w23": g22[:, None] * W23,
        "b23v": (f64(bt22) @ f64(W23) + b23).astype(np.float32),
        "wlt": Wl[:512], "wlb": g23[:, None] * Wl[512:],
        "blv": (f64(bl) + f64(bt23) @ f64(Wl[512:])).astype(np.float32),
        "wh1": gl[:, None] * Wh1,
        "bh1v": (f64(btl) @ f64(Wh1) + bh1).astype(np.float32),
        "wh2": gh1[:, None] * Wh2,
        "bh2v": (f64(bth1) @ f64(Wh2) + bh2).astype(np.float32),
        "whw": gh2[:, None] * WhW,
        "bwv": (f64(bth2) @ f64(WhW) + bhW).astype(np.float32),
    }
    casts = {"wc1", "wb1", "wc2", "wb2", "w22", "w23", "wlt", "wlb",
             "wh1", "wh2", "whw"}
    out = {}
    for k, v in d.items():
        out[k] = np.ascontiguousarray(
            v.astype(np.float16) if k in casts else v.astype(np.float32))
    out["jidx"] = np.arange(512, dtype=np.float32)
    out["ident"] = np.eye(128, dtype=np.float32)
    out["ident16"] = np.eye(128, dtype=np.float16)
    return out, ok


def _reference_numpy(x, params):
    """Exact eval-mode fallback (never used for the shipped seed, where all
    fold-relevant BN scales are positive)."""
    x = np.asarray(x, np.float32)
    B = 8
    Pn = x.shape[0] // B
    xg = x.reshape(B, Pn, -1)

    def mlp(a, layers):
        for (W, b, g, bt) in layers:
            a = np.maximum(a @ np.asarray(W, np.float32) + np.asarray(b, np.float32), 0)
            a = a * np.asarray(g, np.float32) + np.asarray(bt, np.float32)
        return a

    def edge_conv(a, layers):
        x2 = (a * a).sum(-1)
        d2 = x2[:, :, None] + x2[:, None, :] - 2.0 * np.einsum("bic,bjc->bij", a, a)
        idx = np.argsort(d2, axis=2, kind="stable")[:, :, :K]
        out = []
        for bi in range(B):
            xj = a[bi][idx[bi]]
            xi = np.broadcast_to(a[bi][:, None, :], xj.shape)
            feat = np.concatenate([xi, xj - xi], -1)
            out.append(mlp(feat, layers).max(1))
        return np.stack(out)

    x1 = edge_conv(xg, params["conv1"])
    x2_ = edge_conv(x1, params["conv2"])
    o = mlp(np.concatenate([x1, x2_], -1), params["lin1"]).max(1)
    o = mlp(o, params["head1"])
    o = mlp(o, params["head2"])
    return (o @ np.asarray(params["headW"], np.float32)
            + np.asarray(params["headb"], np.float32)).astype(np.float32)


def kernel(x, batch=None, num_graphs=None, params=None, **_):
    x = np.ascontiguousarray(np.asarray(x, np.float32))
    folded, ok = _fold_params(params)
    if not ok:
        return _reference_numpy(x, params)

    nc = _build_program()
    in_maps = []
    for c in range(N_CORES):
        m = dict(folded)
        m["xg"] = x[c * P:(c + 1) * P]
        in_maps.append(m)
    res = run_bass_kernel_spmd(nc, in_maps, list(range(N_CORES)))
    return np.stack([res.results[c]["out"] for c in range(N_CORES)]).astype(np.float32)


if __name__ == "__main__":
    _build_program()
    print("program built + compiled OK")


# revision 16
# speedup vs baseline: 16.2443x; 1.0385x over previous
"""Trainium2 Bass kernel for nn_DynamicModel_79955111182517 (DGCNN-style
EdgeConv GNN, B=8 graphs x P=512 points x C=1024, k=16 kNN).

Strategy (data-parallel, one graph per NeuronCore):
- BatchNorm (eval) scale/shift folded into adjacent matmuls on the host;
  max-aggregations commute with the positive per-channel scales.
- EdgeConv message nn([x_i, x_j - x_i]) is decomposed: feat @ W =
  x_i @ (Wtop - Wbot) + x_j @ Wbot, so only two P x C matmuls are needed
  instead of P*k x 2C.
- conv1 (single-layer MLP) factorizes fully: max_k relu(c_i + b_j) =
  relu(c_i + max_{j in knn(i)} b_j).
- kNN: fp32 Gram matrix on the PE, top-16 per row via the DVE max8 /
  max_index / match_replace ISA ops; neighbor gathers are one-hot fp16
  matmuls on the PE (keeps everything feature-major for the MLP chain).
- All heavy matmuls run in fp16 (full PE rate, ~8x finer rounding than
  bf16); distances in fp32 so neighbor selection matches the reference.
"""

import sys
import numpy as np

try:
    import concourse.bass as bass  # noqa: F401
except ImportError:
    sys.path.insert(0, "/opt/trn_rl_repo")

import concourse.bass as bass
import concourse.tile as tile
from concourse import bacc, mybir
from concourse.bass_utils import run_bass_kernel_spmd

F32 = mybir.dt.float32
F16 = mybir.dt.float16
U32 = mybir.dt.uint32
I16 = mybir.dt.int16
AF = mybir.ActivationFunctionType
OP = mybir.AluOpType

N_CORES = 8
P = 512          # points per graph
C0 = 1024        # input channels
K = 16           # neighbors
NI = P // 128    # 4 token tiles
CH = 512         # gather-chunk columns (32 points x 16 ranks)
NCH = (P * K) // CH  # 16 chunks
NEG = -1.0e30
USE_DMA_GATHER = True

_PROG_CACHE = {}


# --------------------------------------------------------------------------
# device program
# --------------------------------------------------------------------------

def _declare(nc):
    t = {}

    def inp(name, shape, dt):
        t[name] = nc.dram_tensor(name, list(shape), dt, kind="ExternalInput").ap()

    inp("xg", [P, C0], F32)
    inp("wc1", [1024, 512], F16)
    inp("wb1", [1024, 512], F16)
    inp("b1v", [512], F32)
    inp("g1v", [512], F32)
    inp("bt1v", [512], F32)
    inp("wc2", [512, 512], F16)
    inp("wb2", [512, 512], F16)
    inp("b21v", [512], F32)
    inp("w22", [512, 1024], F16)
    inp("b22v", [1024], F32)
    inp("w23", [1024, 1024], F16)
    inp("b23v", [1024], F32)
    inp("wlt", [512, 2048], F16)
    inp("wlb", [1024, 2048], F16)
    inp("blv", [2048], F32)
    inp("wh1", [2048, 1024], F16)
    inp("bh1v", [1024], F32)
    inp("wh2", [1024, 512], F16)
    inp("bh2v", [512], F32)
    inp("whw", [512, 10], F16)
    inp("bwv", [10], F32)
    inp("jidx", [512], F32)         # arange(512) for one-hot compares
    inp("ident", [128, 128], F32)   # PE-transpose identity
    inp("ident16", [128, 128], F16)

    t["out"] = nc.dram_tensor("out", [10], F32, kind="ExternalOutput").ap()
    # internal DRAM bounce buffers (partition -> row reshuffles)
    t["irow1"] = nc.dram_tensor("irow1", [P * K], F16).ap()
    t["irow2"] = nc.dram_tensor("irow2", [P * K], F16).ap()
    t["xrow1"] = nc.dram_tensor("xrow1", [P], F32).ap()
    t["xrow2"] = nc.dram_tensor("xrow2", [P], F32).ap()
    return t


def _make_pools(ctx, tc):
    return dict(
        wa=ctx.enter_context(tc.tile_pool(name="wa", bufs=1)),
        wb=ctx.enter_context(tc.tile_pool(name="wb", bufs=1)),
        sb=ctx.enter_context(tc.tile_pool(name="sb", bufs=1)),
        cols=ctx.enter_context(tc.tile_pool(name="cols", bufs=1)),
        pmm=ctx.enter_context(tc.tile_pool(name="pmm", bufs=6, space="PSUM")),
        ptr=ctx.enter_context(tc.tile_pool(name="ptr", bufs=2, space="PSUM")),
    )


def _emit(ctx, tc, t, pools=None):
    nc = tc.nc
    if pools is None:
        pools = _make_pools(ctx, tc)
    wa, wb, sb, cols, pmm, ptr = (pools[k] for k in
                                  ("wa", "wb", "sb", "cols", "pmm", "ptr"))

    def col_load(vec_ap, n128, base_name, dt=F32):
        out = []
        for i in range(n128):
            n = min(128, vec_ap.shape[0] - i * 128)
            c = cols.tile([n, 1], dt, name=f"{base_name}{i}", tag=f"{base_name}{i}")
            nc.sync.dma_start(out=c, in_=vec_ap[i * 128:i * 128 + n])
            out.append(c)
        return out

    # ---- constant columns ----
    b1c = col_load(t["b1v"], 4, "b1c")
    g1c = col_load(t["g1v"], 4, "g1c")
    bt1c = col_load(t["bt1v"], 4, "bt1c")
    b21c = col_load(t["b21v"], 4, "b21c")
    b22c = col_load(t["b22v"], 8, "b22c")
    b23c = col_load(t["b23v"], 8, "b23c")
    blc = col_load(t["blv"], 16, "blc")
    bh1c = col_load(t["bh1v"], 8, "bh1c")
    bh2c = col_load(t["bh2v"], 4, "bh2c")
    bwc = col_load(t["bwv"], 1, "bwc")
    jc = col_load(t["jidx"], 4, "jc", dt=F32)
    identw = cols.tile([128, 128], F32, name="identw", tag="identw")
    nc.sync.dma_start(out=identw, in_=t["ident"])
    identw16 = cols.tile([128, 128], F16, name="identw16", tag="identw16")
    nc.sync.dma_start(out=identw16, in_=t["ident16"])

    # ==================================================================
    # stage 0/1: load X blockwise, transpose on PE -> XT (fp32 + fp16)
    # ==================================================================
    xt32 = [sb.tile([128, P], F32, name=f"xt32_{ct}", tag=f"xt32_{ct}") for ct in range(8)]
    xt16 = [sb.tile([128, P], F16, name=f"xt16_{ct}", tag=f"xt16_{ct}") for ct in range(8)]
    for ct in range(8):
        for it in range(NI):
            xb = sb.tile([128, 128], F32, name="xb", tag="xblk", bufs=6)
            nc.sync.dma_start(
                out=xb, in_=t["xg"][it * 128:(it + 1) * 128, ct * 128:(ct + 1) * 128])
            pt = ptr.tile([128, 128], F32, name="pt", tag="ptr")
            nc.tensor.transpose(pt, xb, identw)
            nc.scalar.activation(out=xt32[ct][:, it * 128:(it + 1) * 128], in_=pt, func=AF.Copy)
        nc.vector.tensor_copy(out=xt16[ct], in_=xt32[ct])

    # ---- weight slab A: conv1 (later reused for lin1) ----
    # layout: [wc1 8kt x 512 | wb1 8kt x 512]
    slabA = wa.tile([128, 16 * 512], F16, name="slabA", tag="wslabA")
    for kt in range(8):
        nc.scalar.dma_start(out=slabA[:, kt * 512:(kt + 1) * 512],
                          in_=t["wc1"][kt * 128:(kt + 1) * 128, :])
        nc.scalar.dma_start(out=slabA[:, (8 + kt) * 512:(9 + kt) * 512],
                          in_=t["wb1"][kt * 128:(kt + 1) * 128, :])

    def wA1(kt, mt):  # wc1 lhsT block [128k, 128m]
        return slabA[:, kt * 512 + mt * 128: kt * 512 + (mt + 1) * 128]

    def wA2(kt):      # wb1 rhs k-tile [128k, 512]
        return slabA[:, (8 + kt) * 512:(9 + kt) * 512]

    # ---- weight slab B: conv2 (later reused for heads) ----
    # layout: [wc2 4x512 | wb2 4x512 | w22 4x1024 | w23 8x1024]
    slabB = wb.tile([128, 2048 + 2048 + 4096 + 8192], F16, name="slabB", tag="wslabB")
    oB2 = 2048
    oB3 = 4096
    oB4 = 8192
    for kt in range(4):
        nc.scalar.dma_start(out=slabB[:, kt * 512:(kt + 1) * 512],
                          in_=t["wc2"][kt * 128:(kt + 1) * 128, :])
        nc.scalar.dma_start(out=slabB[:, oB2 + kt * 512:oB2 + (kt + 1) * 512],
                          in_=t["wb2"][kt * 128:(kt + 1) * 128, :])
        nc.scalar.dma_start(out=slabB[:, oB3 + kt * 1024:oB3 + (kt + 1) * 1024],
                          in_=t["w22"][kt * 128:(kt + 1) * 128, :])
    for kt in range(8):
        nc.scalar.dma_start(out=slabB[:, oB4 + kt * 1024:oB4 + (kt + 1) * 1024],
                          in_=t["w23"][kt * 128:(kt + 1) * 128, :])

    def wB_c2(kt, mt):
        return slabB[:, kt * 512 + mt * 128: kt * 512 + (mt + 1) * 128]

    def wB_b2(kt):
        return slabB[:, oB2 + kt * 512:oB2 + (kt + 1) * 512]

    def wB_22(kt, nt):
        return slabB[:, oB3 + kt * 1024 + nt * 128: oB3 + kt * 1024 + (nt + 1) * 128]

    def wB_23(kt, nt):
        return slabB[:, oB4 + kt * 1024 + nt * 128: oB4 + kt * 1024 + (nt + 1) * 128]

    # ==================================================================
    # generic helpers
    # ==================================================================
    def gram_topk_idxrow(xt_f32, nk, gsb_tag, irow_dram, xrow_dram, conv_id):
        """Gram matrix (fp32), scores s = 2G - x2_j, top-16 indices per row,
        assembled into a broadcast index row [128, P*K] (f16 values)."""
        gsb = []
        for mt in range(NI):
            pg = pmm.tile([128, P], F32, name="pg", tag="pmm")
            for kt in range(nk):
                nc.tensor.matmul(pg, lhsT=xt_f32[kt][:, mt * 128:(mt + 1) * 128],
                                 rhs=xt_f32[kt], start=(kt == 0), stop=(kt == nk - 1))
            g = sb.tile([128, P], F32, name=f"g{conv_id}_{mt}", tag=f"{gsb_tag}{mt}")
            nc.scalar.activation(out=g, in_=pg, func=AF.Copy)
            gsb.append(g)

        # x2 row from the diagonal of G
        for mt in range(NI):
            dsel = sb.tile([128, P], F32, name="dsel", tag="dsel", bufs=2)
            nc.gpsimd.affine_select(
                out=dsel, in_=gsb[mt], compare_op=OP.is_equal, fill=0.0,
                base=128 * mt, pattern=[[-1, P]], channel_multiplier=1)
            dcol = sb.tile([128, 1], F32, name="dcol", tag="dcol", bufs=2)
            nc.vector.reduce_sum(out=dcol, in_=dsel, axis=mybir.AxisListType.X)
            nc.sync.dma_start(out=xrow_dram[mt * 128:(mt + 1) * 128], in_=dcol)
        x2b = sb.tile([128, P], F32, name=f"x2b{conv_id}", tag="x2b")
        nc.sync.dma_start(out=x2b, in_=xrow_dram.rearrange("(o n) -> o n", o=1).to_broadcast([128, P]))

        idxg = None
        if USE_DMA_GATHER:
            idxg = sb.tile([128, P], I16, name=f"idxg{conv_id}", tag="idxg", bufs=2)
        # s = 2G - x2_j (in place over gsb); then top-16 via max8/match_replace
        for mt in range(NI):
            nc.vector.scalar_tensor_tensor(
                out=gsb[mt], in0=gsb[mt], scalar=2.0, in1=x2b,
                op0=OP.mult, op1=OP.subtract)
            va = sb.tile([128, 8], F32, name="va", tag="va", bufs=2)
            vb = sb.tile([128, 8], F32, name="vb", tag="vb", bufs=2)
            idxu = sb.tile([128, K], U32, name="idxu", tag="idxu", bufs=2)
            idxf = None
            if not USE_DMA_GATHER:
                idxf = sb.tile([128, K], F16, name="idxf", tag="idxf", bufs=2)
            nc.vector.max(out=va, in_=gsb[mt])
            nc.vector.max_index(out=idxu[:, 0:8], in_max=va, in_values=gsb[mt])
            nc.vector.match_replace(out=gsb[mt], in_to_replace=va,
                                    in_values=gsb[mt], imm_value=NEG)
            nc.vector.max(out=vb, in_=gsb[mt])
            nc.vector.max_index(out=idxu[:, 8:16], in_max=vb, in_values=gsb[mt])
            if USE_DMA_GATHER:
                idx16 = sb.tile([128, K], F16, name="idx16", tag="idx16", bufs=2)
                nc.vector.tensor_copy(out=idx16, in_=idxu)
                pti = ptr.tile([K, 128], F16, name="pti", tag="ptr")
                nc.tensor.transpose(pti, idx16, identw16)
                nc.vector.tensor_copy(
                    out=idxg[0:K, mt * 128:(mt + 1) * 128], in_=pti)
            else:
                nc.vector.tensor_copy(out=idxf, in_=idxu)
                nc.sync.dma_start(out=irow_dram[mt * 128 * K:(mt + 1) * 128 * K], in_=idxf)

        if USE_DMA_GATHER:
            for g in range(1, 8):
                nc.sync.dma_start(out=idxg[g * K:(g + 1) * K, :], in_=idxg[0:K, :])
            return idxg
        irow_b = sb.tile([128, P * K], F16, name=f"irow_b{conv_id}", tag="irow_b")
        nc.sync.dma_start(out=irow_b, in_=irow_dram.rearrange("(o n) -> o n", o=1).to_broadcast([128, P * K]))
        return irow_b

    def onehot_chunk(irow_b, ch, conv_id):
        ohs = []
        for jt in range(NI):
            oh = sb.tile([128, CH], F16, name="oh", tag=f"oh{jt}", bufs=2)
            nc.vector.tensor_scalar(
                oh, irow_b[:, ch * CH:(ch + 1) * CH], jc[jt], None, op0=OP.is_equal)
            ohs.append(oh)
        return ohs

    # ==================================================================
    # conv1
    # ==================================================================
    irow_b1 = gram_topk_idxrow(xt32, 8, "gsb", t["irow1"], t["xrow1"], 1)

    btok1 = sb.tile([128, NI * 512], F16, name="btok1", tag="btok")
    for jt in range(NI):
        pb = pmm.tile([128, 512], F32, name="pb", tag="pmm")
        for kt in range(8):
            nc.tensor.matmul(pb, lhsT=xt16[kt][:, jt * 128:(jt + 1) * 128],
                             rhs=wA2(kt), start=(kt == 0), stop=(kt == 7))
        nc.vector.tensor_copy(out=btok1[:, jt * 512:(jt + 1) * 512], in_=pb)

    # c1 = X @ (Wtop-Wbot) + b1  (feature-major), b1tok = X @ Wbot (token-major)
    c1T = []
    for mt in range(NI):
        pc = pmm.tile([128, P], F32, name="pc", tag="pmm")
        for kt in range(8):
            nc.tensor.matmul(pc, lhsT=wA1(kt, mt), rhs=xt16[kt],
                             start=(kt == 0), stop=(kt == 7))
        c = sb.tile([128, P], F32, name=f"c1T{mt}", tag=f"cT{mt}")
        nc.vector.tensor_scalar(c, pc, b1c[mt], None, op0=OP.add)
        c1T.append(c)
    # gather chunks: T[c, i] = max_r b[knn_r(i), c]
    Tt = [sb.tile([128, P], F32, name=f"Tt{ct}", tag=f"xt32_{ct}") for ct in range(NI)]
    IC1 = CH // K
    GCH = 512                    # gather width (columns)
    GIC = GCH // K
    for ch in range(NCH if not USE_DMA_GATHER else (P * K) // GCH):
        if USE_DMA_GATHER:
            gout = sb.tile([128, NI, GCH], F16, name="gout1", tag="gout", bufs=3)
            nc.gpsimd.dma_gather(
                out_ap=gout, in_ap=btok1,
                idxs_ap=irow_b1[:, ch * GIC:(ch + 1) * GIC],
                num_idxs=GCH, num_idxs_reg=GCH, elem_size=512,
                transpose=True, queue_num=0,
                sbuf_tokens_per_rank=128, sbuf_free_dim_per_rank=1024)
            for ct in range(NI):
                nc.vector.tensor_reduce(
                    out=Tt[ct][:, ch * GIC:(ch + 1) * GIC],
                    in_=gout[:, ct, :].rearrange("p (i r) -> p i r", r=K),
                    axis=mybir.AxisListType.X, op=OP.max)
        else:
            ohs = onehot_chunk(irow_b1, ch, 1)
            for ct in range(NI):
                pgt = pmm.tile([128, CH], F32, name="pgt", tag="pmm")
                for jt in range(NI):
                    nc.tensor.matmul(pgt, lhsT=btok1[:, jt * 512 + ct * 128:jt * 512 + (ct + 1) * 128],
                                     rhs=ohs[jt], start=(jt == 0), stop=(jt == NI - 1))
                nc.vector.tensor_reduce(
                    out=Tt[ct][:, ch * IC1:(ch + 1) * IC1],
                    in_=pgt.rearrange("p (i r) -> p i r", r=K),
                    axis=mybir.AxisListType.X, op=OP.max)

    # x1 = g1 * relu(c1 + T) + bt1   (keep fp32 for Gram, fp16 for matmuls)
    x1_32 = []
    x1_16 = []
    for ct in range(NI):
        nc.vector.tensor_add(c1T[ct], c1T[ct], Tt[ct])
        x32 = sb.tile([128, P], F32, name=f"x1_32_{ct}", tag=f"xt32_{4 + ct}")
        nc.scalar.activation(out=x32, in_=c1T[ct], func=AF.Relu)
        nc.vector.tensor_scalar(x32, x32, g1c[ct], bt1c[ct], op0=OP.mult, op1=OP.add)
        x16 = sb.tile([128, P], F16, name=f"x1_16_{ct}", tag=f"x1_16_{ct}")
        nc.vector.tensor_copy(out=x16, in_=x32)
        x1_32.append(x32)
        x1_16.append(x16)

    # ==================================================================
    # conv2
    # ==================================================================
    irow_b2 = gram_topk_idxrow(x1_32, NI, "gsb", t["irow2"], t["xrow2"], 2)

    btok2 = sb.tile([128, NI * 512], F16, name="btok2", tag="btok")
    for jt in range(NI):
        pb2 = pmm.tile([128, 512], F32, name="pb2", tag="pmm")
        for kt in range(NI):
            nc.tensor.matmul(pb2, lhsT=x1_16[kt][:, jt * 128:(jt + 1) * 128],
                             rhs=wB_b2(kt), start=(kt == 0), stop=(kt == NI - 1))
        nc.vector.tensor_copy(out=btok2[:, jt * 512:(jt + 1) * 512], in_=pb2)

    c2T = []
    for mt in range(NI):
        pc2 = pmm.tile([128, P], F32, name="pc2", tag="pmm")
        for kt in range(NI):
            nc.tensor.matmul(pc2, lhsT=wB_c2(kt, mt), rhs=x1_16[kt],
                             start=(kt == 0), stop=(kt == NI - 1))
        c2 = sb.tile([128, P], F32, name=f"c2T{mt}", tag=f"cT{mt}")
        nc.vector.tensor_scalar(c2, pc2, b21c[mt], None, op0=OP.add)
        c2T.append(c2)
    x2acc = [sb.tile([128, P], F32, name=f"x2acc{nt}", tag=f"xt32_{nt}") for nt in range(8)]
    IC = CH // K  # 32 points per chunk
    for ch in range(NCH):
        h1 = []
        if USE_DMA_GATHER:
            gout2 = sb.tile([128, NI, CH], F16, name="gout2", tag="gout", bufs=3)
            nc.gpsimd.dma_gather(
                out_ap=gout2, in_ap=btok2,
                idxs_ap=irow_b2[:, ch * IC:(ch + 1) * IC],
                num_idxs=CH, num_idxs_reg=CH, elem_size=512,
                transpose=True, queue_num=0,
                sbuf_tokens_per_rank=128, sbuf_free_dim_per_rank=1024)
            for ct in range(NI):
                h = sb.tile([128, CH], F16, name="h1", tag=f"h1_{ct}", bufs=2)
                cbc = c2T[ct][:, ch * IC:(ch + 1) * IC].rearrange(
                    "p (i o) -> p i o", o=1).to_broadcast([128, IC, K])
                nc.vector.tensor_tensor(
                    out=h.rearrange("p (i r) -> p i r", r=K),
                    in0=gout2[:, ct, :].rearrange("p (i r) -> p i r", r=K),
                    in1=cbc, op=OP.add)
                nc.scalar.activation(out=h, in_=h, func=AF.Relu)
                h1.append(h)
        else:
            ohs = onehot_chunk(irow_b2, ch, 2)
            for ct in range(NI):
                pg2 = pmm.tile([128, CH], F32, name="pg2", tag="pmm")
                for jt in range(NI):
                    nc.tensor.matmul(pg2, lhsT=btok2[:, jt * 512 + ct * 128:jt * 512 + (ct + 1) * 128],
                                     rhs=ohs[jt], start=(jt == 0), stop=(jt == NI - 1))
                h = sb.tile([128, CH], F16, name="h1", tag=f"h1_{ct}", bufs=2)
                cbc = c2T[ct][:, ch * IC:(ch + 1) * IC].rearrange(
                    "p (i o) -> p i o", o=1).to_broadcast([128, IC, K])
                nc.vector.tensor_tensor(
                    out=h.rearrange("p (i r) -> p i r", r=K),
                    in0=pg2.rearrange("p (i r) -> p i r", r=K),
                    in1=cbc, op=OP.add)
                nc.scalar.activation(out=h, in_=h, func=AF.Relu)
                h1.append(h)
        # layer2: h2 = relu(h1 @ W22f + b22f)
        h2 = []
        for nt in range(8):
            pz2 = pmm.tile([128, CH], F32, name="pz2", tag="pmm")
            for kt in range(NI):
                nc.tensor.matmul(pz2, lhsT=wB_22(kt, nt), rhs=h1[kt],
                                 start=(kt == 0), stop=(kt == NI - 1))
            hh = sb.tile([128, CH], F16, name="h2", tag=f"h2_{nt}", bufs=2)
            nc.scalar.activation(out=hh, in_=pz2, func=AF.Relu, bias=b22c[nt])
            h2.append(hh)
        # layer3 + max over ranks
        for nt in range(8):
            pz3 = pmm.tile([128, CH], F32, name="pz3", tag="pmm")
            for kt in range(8):
                nc.tensor.matmul(pz3, lhsT=wB_23(kt, nt), rhs=h2[kt],
                                 start=(kt == 0), stop=(kt == 7))
            nc.vector.tensor_reduce(
                out=x2acc[nt][:, ch * IC:(ch + 1) * IC],
                in_=pz3.rearrange("p (i r) -> p i r", r=K),
                axis=mybir.AxisListType.X, op=OP.max)

    # x2relu = relu(x2acc + b23f) -> fp16
    x2r = []
    for nt in range(8):
        xx = sb.tile([128, P], F16, name=f"x2r{nt}", tag=f"x2r{nt}")
        nc.scalar.activation(out=xx, in_=x2acc[nt], func=AF.Relu, bias=b23c[nt])
        x2r.append(xx)

    # ---- weight slab C: lin1 (reuses slab A space) ----
    slabC = wa.tile([128, 12 * 2048], F16, name="slabC", tag="wslabA")
    for kt in range(4):
        nc.scalar.dma_start(out=slabC[:, kt * 2048:(kt + 1) * 2048],
                          in_=t["wlt"][kt * 128:(kt + 1) * 128, :])
    for kt in range(8):
        nc.scalar.dma_start(out=slabC[:, (4 + kt) * 2048:(5 + kt) * 2048],
                          in_=t["wlb"][kt * 128:(kt + 1) * 128, :])

    # lin1 + global max pool -> pooled [2048] (fp16 cols)
    pooled = []
    for mt in range(16):
        pzl = pmm.tile([128, P], F32, name="pzl", tag="pmm")
        for kt in range(4):
            nc.tensor.matmul(
                pzl, lhsT=slabC[:, kt * 2048 + mt * 128: kt * 2048 + (mt + 1) * 128],
                rhs=x1_16[kt], start=(kt == 0), stop=False)
        for kt in range(8):
            nc.tensor.matmul(
                pzl, lhsT=slabC[:, (4 + kt) * 2048 + mt * 128: (4 + kt) * 2048 + (mt + 1) * 128],
                rhs=x2r[kt], start=False, stop=(kt == 7))
        pcol = sb.tile([128, 1], F32, name="pcol", tag="pcol", bufs=4)
        nc.vector.tensor_reduce(out=pcol, in_=pzl, axis=mybir.AxisListType.X, op=OP.max)
        pc16 = sb.tile([128, 1], F16, name=f"pool{mt}", tag=f"pool{mt}")
        nc.scalar.activation(out=pc16, in_=pcol, func=AF.Relu, bias=blc[mt])
        pooled.append(pc16)

    # ---- weight slab D: heads (reuses slab B space) ----
    slabD = wb.tile([128, 16 * 1024 + 8 * 512 + 4 * 16], F16, name="slabD", tag="wslabB")
    oD2 = 16 * 1024
    oD3 = oD2 + 8 * 512
    for kt in range(16):
        nc.scalar.dma_start(out=slabD[:, kt * 1024:(kt + 1) * 1024],
                          in_=t["wh1"][kt * 128:(kt + 1) * 128, :])
    for kt in range(8):
        nc.scalar.dma_start(out=slabD[:, oD2 + kt * 512:oD2 + (kt + 1) * 512],
                          in_=t["wh2"][kt * 128:(kt + 1) * 128, :])
    for kt in range(4):
        nc.scalar.dma_start(out=slabD[:, oD3 + kt * 16:oD3 + kt * 16 + 10],
                          in_=t["whw"][kt * 128:(kt + 1) * 128, :])

    # head1
    hh1 = []
    for mt in range(8):
        ph1 = ptr.tile([128, 1], F32, name="ph1", tag="ptr")
        for kt in range(16):
            nc.tensor.matmul(
                ph1, lhsT=slabD[:, kt * 1024 + mt * 128: kt * 1024 + (mt + 1) * 128],
                rhs=pooled[kt], start=(kt == 0), stop=(kt == 15))
        h1c = sb.tile([128, 1], F16, name=f"hh1_{mt}", tag=f"hh1_{mt}")
        nc.scalar.activation(out=h1c, in_=ph1, func=AF.Relu, bias=bh1c[mt])
        hh1.append(h1c)
    # head2
    hh2 = []
    for mt in range(4):
        ph2 = ptr.tile([128, 1], F32, name="ph2", tag="ptr")
        for kt in range(8):
            nc.tensor.matmul(
                ph2, lhsT=slabD[:, oD2 + kt * 512 + mt * 128: oD2 + kt * 512 + (mt + 1) * 128],
                rhs=hh1[kt], start=(kt == 0), stop=(kt == 7))
        h2c = sb.tile([128, 1], F16, name=f"hh2_{mt}", tag=f"hh2_{mt}")
        nc.scalar.activation(out=h2c, in_=ph2, func=AF.Relu, bias=bh2c[mt])
        hh2.append(h2c)
    # final linear
    po = ptr.tile([10, 1], F32, name="po", tag="ptr")
    for kt in range(4):
        nc.tensor.matmul(po, lhsT=slabD[:, oD3 + kt * 16:oD3 + kt * 16 + 10],
                         rhs=hh2[kt], start=(kt == 0), stop=(kt == 3))
    osb = sb.tile([10, 1], F32, name="osb", tag="osb")
    nc.vector.tensor_scalar(osb, po, bwc[0][:10], None, op0=OP.add)
    nc.sync.dma_start(out=t["out"], in_=osb)


def _build_program():
    if "prog" in _PROG_CACHE:
        return _PROG_CACHE["prog"]
    from contextlib import ExitStack
    nc = bacc.Bacc("TRN2", target_bir_lowering=False, debug=False,
                   num_devices=N_CORES)
    t = _declare(nc)
    with tile.TileContext(nc) as tc:
        with ExitStack() as ctx:
            _emit(ctx, tc, t)
    nc.compile()
    _PROG_CACHE["prog"] = nc
    return nc


# --------------------------------------------------------------------------
# host side
# --------------------------------------------------------------------------

def _fold_params(params):
    def f32(a):
        return np.asarray(a, np.float32)

    def f64(a):
        return np.asarray(a, np.float64)

    (W1, b1, g1, bt1), = [tuple(map(f32, l)) for l in params["conv1"]]
    conv2 = [tuple(map(f32, l)) for l in params["conv2"]]
    (W21, b21, g21, bt21), (W22, b22, g22, bt22), (W23, b23, g23, bt23) = conv2
    (Wl, bl, gl, btl), = [tuple(map(f32, l)) for l in params["lin1"]]
    (Wh1, bh1, gh1, bth1), = [tuple(map(f32, l)) for l in params["head1"]]
    (Wh2, bh2, gh2, bth2), = [tuple(map(f32, l)) for l in params["head2"]]
    WhW = f32(params["headW"])
    bhW = f32(params["headb"])

    ok = min(g1.min(), g23.min(), gl.min()) > 0
    d = {
        "wc1": W1[:1024] - W1[1024:], "wb1": W1[1024:],
        "b1v": b1, "g1v": g1, "bt1v": bt1,
        "wc2": W21[:512] - W21[512:], "wb2": W21[512:], "b21v": b21,
        "w22": g21[:, None] * W22,
        "b22v": (f64(bt21) @ f64(W22) + b22).astype(np.float32),
        "w23": g22[:, None] * W23,
        "b23v": (f64(bt22) @ f64(W23) + b23).astype(np.float32),
        "wlt": Wl[:512], "wlb": g23[:, None] * Wl[512:],
        "blv": (f64(bl) + f64(bt23) @ f64(Wl[512:])).astype(np.float32),
        "wh1": gl[:, None] * Wh1,
        "bh1v": (f64(btl) @ f64(Wh1) + bh1).astype(np.float32),
        "wh2": gh1[:, None] * Wh2,
        "bh2v": (f64(bth1) @ f64(Wh2) + bh2).astype(np.float32),
        "whw": gh2[:, None] * WhW,
        "bwv": (f64(bth2) @ f64(WhW) + bhW).astype(np.float32),
    }
    casts = {"wc1", "wb1", "wc2", "wb2", "w22", "w23", "wlt", "wlb",
             "wh1", "wh2", "whw"}
    out = {}
    for k, v in d.items():
        out[k] = np.ascontiguousarray(
            v.astype(np.float16) if k in casts else v.astype(np.float32))
    out["jidx"] = np.arange(512, dtype=np.float32)
    out["ident"] = np.eye(128, dtype=np.float32)
    out["ident16"] = np.eye(128, dtype=np.float16)
    return out, ok


def _reference_numpy(x, params):
    """Exact eval-mode fallback (never used for the shipped seed, where all
    fold-relevant BN scales are positive)."""
    x = np.asarray(x, np.float32)
    B = 8
    Pn = x.shape[0] // B
    xg = x.reshape(B, Pn, -1)

    def mlp(a, layers):
        for (W, b, g, bt) in layers:
            a = np.maximum(a @ np.asarray(W, np.float32) + np.asarray(b, np.float32), 0)
            a = a * np.asarray(g, np.float32) + np.asarray(bt, np.float32)
        return a

    def edge_conv(a, layers):
        x2 = (a * a).sum(-1)
        d2 = x2[:, :, None] + x2[:, None, :] - 2.0 * np.einsum("bic,bjc->bij", a, a)
        idx = np.argsort(d2, axis=2, kind="stable")[:, :, :K]
        out = []
        for bi in range(B):
            xj = a[bi][idx[bi]]
            xi = np.broadcast_to(a[bi][:, None, :], xj.shape)
            feat = np.concatenate([xi, xj - xi], -1)
            out.append(mlp(feat, layers).max(1))
        return np.stack(out)

    x1 = edge_conv(xg, params["conv1"])
    x2_ = edge_conv(x1, params["conv2"])
    o = mlp(np.concatenate([x1, x2_], -1), params["lin1"]).max(1)
    o = mlp(o, params["head1"])
    o = mlp(o, params["head2"])
    return (o @ np.asarray(params["headW"], np.float32)
            + np.asarray(params["headb"], np.float32)).astype(np.float32)


def kernel(x, batch=None, num_graphs=None, params=None, **_):
    x = np.ascontiguousarray(np.asarray(x, np.float32))
    folded, ok = _fold_params(params)
    if not ok:
        return _reference_numpy(x, params)

    nc = _build_program()
    in_maps = []
    for c in range(N_CORES):
        m = dict(folded)
        m["xg"] = x[c * P:(c + 1) * P]
        in_maps.append(m)
    res = run_bass_kernel_spmd(nc, in_maps, list(range(N_CORES)))
    return np.stack([res.results[c]["out"] for c in range(N_CORES)]).astype(np.float32)


if __name__ == "__main__":
    _build_program()
    print("program built + compiled OK")
